# revision 3
# baseline (speedup 1.0000x reference)
"""Trainium2 Bass kernel for an 8-level circular DWT (forward + inverse).

Problem: reference computes an 8-level periodized DWT (8-tap filters derived
from `scaling`), returns (denoised, concat(coeffs)).  The inverse transform
is applied with no thresholding, so for orthonormal QMF filters (the DB4
filters the reference ships) reconstruction is exactly the identity:
denoised == x.  The kernel verifies the orthonormality condition numerically
and short-circuits the inverse to a host-side copy; the forward transform
(the real work) runs on 8 NeuronCores, data-parallel over rows.

Device algorithm per level (length n per row, filter f, circular):
    out[j] = sum_{k=0..7} f[k] * x[(2j - k) mod n],   j in [0, n/2)
computed as 3 PSUM-accumulated 128x128 matmuls per 128-output block against
banded stationary matrices, with the sequence dimension laid out
[p = seq mod 128] down partitions:
    X[p, r, 1+b] = x_r[128b + p],  X[p, r, 0] = halo (last block)
    out[:, c] = W1.T @ X[:,r,1+2c] + W2.T @ X[:,r,2+2c] + Wc.T @ X[:,r,2c]
The approx output lands in PSUM already in next level's layout, so the
PSUM->SBUF copy is partition-aligned; detail outputs stage through SBUF and
DMA out in a [p, r, c] blocked layout that the host untransposes.
"""

import sys
from contextlib import ExitStack

for _p in ("/opt/trn_rl_repo", "/root/.axon_site/_ro/trn_rl_repo"):
    if _p not in sys.path:
        sys.path.append(_p)

import numpy as np

import concourse.bacc as bacc
import concourse.mybir as mybir
import concourse.tile as tile
from concourse.bass_utils import run_bass_kernel_spmd

F32 = mybir.dt.float32
F32R = mybir.dt.float32r

N_ROWS = 512          # total rows
N0 = 65536            # row length (power of two: reference pad is a no-op)
LEVELS = 8
N_CORES = 8
ROWS = N_ROWS // N_CORES  # rows per core
NB0 = N0 // 128


# ----------------------------- host-side math -----------------------------

def _wavelet(s):
    g = s[::-1].copy()
    sign = np.where(np.arange(s.shape[-1]) % 2 == 1, -1.0, 1.0).astype(g.dtype)
    return g * sign


def _make_stationaries(f):
    W = np.zeros((3, 128, 128), dtype=np.float32)
    for q in range(128):
        for k in range(8):
            i = 2 * q - k
            if 0 <= i < 128:
                W[0, i, q] = f[k]
            elif i >= 128:
                W[1, i - 128, q] = f[k]
            else:
                W[2, i + 128, q] = f[k]
    return W


def _make_wmat(scaling):
    mats = []
    for lvl in range(LEVELS):
        s = np.asarray(scaling[lvl], dtype=np.float32)
        mats.append(_make_stationaries(_wavelet(s)))
        mats.append(_make_stationaries(s))
    allw = np.concatenate(mats, axis=0)  # [LEVELS*6, 128, 128]
    return np.ascontiguousarray(allw.transpose(1, 0, 2).reshape(128, -1))


def _round_f32r(arr):
    """Round fp32 to the nearest FP32R value (1s/8e/11m; low 12 bits zero).

    Matches hardware fp32->fp32r downconversion (round-to-nearest-even).
    """
    u = np.ascontiguousarray(arr, dtype=np.float32).view(np.uint32)
    r = (u + 0x7FF + ((u >> 12) & 1)) & np.uint32(0xFFFFF000)
    return r.view(np.float32)


def _pack_x_shard(x_rows):
    rows, n = x_rows.shape
    nb = n // 128
    blocks = x_rows.reshape(rows, nb, 128).transpose(2, 0, 1)  # [p, r, b]
    xt = np.empty((128, rows, nb + 1), dtype=np.float32)
    xt[:, :, 1:] = blocks
    xt[:, :, 0] = blocks[:, :, nb - 1]
    return np.ascontiguousarray(xt.reshape(128, rows * (nb + 1)))


def _unpack_blocks(arr, rows):
    nob = arr.shape[1] // rows
    return arr.reshape(128, rows, nob).transpose(1, 2, 0).reshape(rows, nob * 128)


def _is_orthonormal_qmf(scaling):
    """Perfect reconstruction holds iff every level's scaling filter has
    orthonormal even shifts: sum_k s[k] s[k+2m] == delta(m)."""
    s = np.asarray(scaling, dtype=np.float64)
    if s.shape != (LEVELS, 8):
        return False
    for lvl in range(LEVELS):
        f = s[lvl]
        for m in range(4):
            v = np.dot(f[: 8 - 2 * m], f[2 * m:])
            if abs(v - (1.0 if m == 0 else 0.0)) > 1e-4:
                return False
    return True


def _dwt_backward_numpy(ds, a, scaling):
    """Fallback inverse transform (float64 FFT) for non-orthonormal filters."""
    a = np.asarray(a, dtype=np.float64)
    for lvl in reversed(range(LEVELS)):
        s = np.asarray(scaling[lvl], dtype=np.float64)
        w = _wavelet(s)
        d = np.asarray(ds[lvl], dtype=np.float64)
        n = d.shape[-1] * 2
        fd = np.zeros((d.shape[0], n))
        fd[:, ::2] = d
        fa = np.zeros((a.shape[0], n))
        fa[:, ::2] = a
        a = (np.fft.irfft(np.fft.rfft(fd, axis=-1)
                          * np.conj(np.fft.rfft(w, n=n)), n=n, axis=-1)
             + np.fft.irfft(np.fft.rfft(fa, axis=-1)
                            * np.conj(np.fft.rfft(s, n=n)), n=n, axis=-1))
    return a


# ----------------------------- device kernel ------------------------------

def _build_dwt(tc, xt, wmat, d_outs, a_out):
    nc = tc.nc
    with ExitStack() as ctx:
        wpool = ctx.enter_context(tc.tile_pool(name="wpool", bufs=1))
        x0pool = ctx.enter_context(tc.tile_pool(name="x0pool", bufs=3))
        xpool = ctx.enter_context(tc.tile_pool(name="xpool", bufs=1))
        stpool = ctx.enter_context(tc.tile_pool(name="stpool", bufs=4))
        pdpool = ctx.enter_context(tc.tile_pool(name="pdpool", bufs=2, space="PSUM"))
        papool = ctx.enter_context(tc.tile_pool(name="papool", bufs=2, space="PSUM"))

        W = wpool.tile([128, LEVELS * 6 * 128], F32R, name="Wsb")
        nc.sync.dma_start(W[:], wmat[:])

        xt3 = xt.rearrange("p (r b) -> p r b", b=NB0 + 1)

        Xs = {}
        for lvl in range(1, LEVELS):
            nb = (N0 >> lvl) // 128
            Xs[lvl] = xpool.tile([128, ROWS, nb + 1], F32R, name=f"X{lvl}",
                                 tag=f"X{lvl}")

        for lvl in range(LEVELS):
            nb = (N0 >> lvl) // 128
            nob = nb // 2
            nr = min(ROWS, max(1, 512 // nob))
            nchunks = ROWS // nr
            dh = d_outs[lvl].rearrange("p (r c) -> p r c", c=nob)
            last = lvl + 1 == LEVELS
            if last:
                ah = a_out.rearrange("p (r c) -> p r c", c=nob)

            for ch in range(nchunks):
                r0 = ch * nr
                if lvl == 0:
                    Xc = x0pool.tile([128, nr, nb + 1], F32R, tag="x0t", name="x0t")
                    nc.sync.dma_start(Xc[:], xt3[:, r0:r0 + nr, :])
                    rs = slice(0, nr)
                else:
                    Xc = Xs[lvl]
                    rs = slice(r0, r0 + nr)

                rhs1 = Xc[:, rs, 1:1 + 2 * nob:2]
                rhs2 = Xc[:, rs, 2:2 * nob + 1:2]
                rhsc = Xc[:, rs, 0:2 * nob:2]

                pd = pdpool.tile([128, nr, nob], F32, tag="pd", name="pd")
                pa = papool.tile([128, nr, nob], F32, tag="pa", name="pa")
                for ps, wbase in ((pd, 0), (pa, 3)):
                    k0 = (lvl * 6 + wbase) * 128
                    nc.tensor.matmul(ps[:], W[:, k0:k0 + 128],
                                     rhs1, start=True, stop=False)
                    nc.tensor.matmul(ps[:], W[:, k0 + 128:k0 + 256],
                                     rhs2, start=False, stop=False)
                    nc.tensor.matmul(ps[:], W[:, k0 + 256:k0 + 384],
                                     rhsc, start=False, stop=True)

                st = stpool.tile([128, nr, nob], F32, tag="st", name="st")
                nc.scalar.copy(st[:], pd[:])
                nc.scalar.dma_start(dh[:, r0:r0 + nr, :], st[:])

                if not last:
                    Xn = Xs[lvl + 1]
                    nc.vector.tensor_copy(Xn[:, r0:r0 + nr, 1:1 + nob], pa[:])
                    nc.vector.tensor_copy(Xn[:, r0:r0 + nr, 0:1],
                                          pa[:, :, nob - 1:nob])
                else:
                    sta = stpool.tile([128, nr, nob], F32, tag="st", name="sta")
                    nc.scalar.copy(sta[:], pa[:])
                    nc.scalar.dma_start(ah[:, r0:r0 + nr, :], sta[:])


_MODULE_CACHE = {}


def _get_module():
    if "nc" in _MODULE_CACHE:
        return _MODULE_CACHE["nc"]
    nc = bacc.Bacc("TRN2", target_bir_lowering=False, debug=False,
                   num_devices=N_CORES)
    xt = nc.dram_tensor("xt", [128, ROWS * (NB0 + 1)], F32R,
                        kind="ExternalInput").ap()
    wmat = nc.dram_tensor("wmat", [128, LEVELS * 6 * 128], F32R,
                          kind="ExternalInput").ap()
    d_outs = []
    for lvl in range(LEVELS):
        nob = (N0 >> lvl) // 256
        d_outs.append(nc.dram_tensor(f"d{lvl}", [128, ROWS * nob], F32,
                                     kind="ExternalOutput").ap())
    a_out = nc.dram_tensor("aF", [128, ROWS * ((N0 >> (LEVELS - 1)) // 256)],
                           F32, kind="ExternalOutput").ap()
    with tile.TileContext(nc) as tc:
        _build_dwt(tc, xt, wmat, d_outs, a_out)
    nc.compile()
    _MODULE_CACHE["nc"] = nc
    return nc


def run(x, scaling, **spmd_kwargs):
    """Full pipeline.  Returns (denoised, coeffs, BassKernelResults)."""
    x = np.ascontiguousarray(np.asarray(x, dtype=np.float32))
    scaling = np.asarray(scaling, dtype=np.float32)
    assert x.shape == (N_ROWS, N0), x.shape
    assert scaling.shape == (LEVELS, 8), scaling.shape

    nc = _get_module()
    wmat = _round_f32r(_make_wmat(scaling))
    in_maps = []
    for c in range(N_CORES):
        in_maps.append({
            "xt": _pack_x_shard(_round_f32r(x[c * ROWS:(c + 1) * ROWS])),
            "wmat": wmat,
        })

    res = run_bass_kernel_spmd(nc, in_maps, core_ids=list(range(N_CORES)),
                               **spmd_kwargs)

    coeffs = np.empty((N_ROWS, N0), dtype=np.float32)
    off = 0
    ds_full = []
    for lvl in range(LEVELS):
        half = (N0 >> lvl) // 2
        dcols = coeffs[:, off:off + half]
        for c in range(N_CORES):
            dcols[c * ROWS:(c + 1) * ROWS] = _unpack_blocks(
                res.results[c][f"d{lvl}"], ROWS)
        ds_full.append(dcols)
        off += half
    a_full = np.empty((N_ROWS, N0 - off), dtype=np.float32)
    for c in range(N_CORES):
        a_full[c * ROWS:(c + 1) * ROWS] = _unpack_blocks(
            res.results[c]["aF"], ROWS)
    coeffs[:, off:] = a_full

    if _is_orthonormal_qmf(scaling):
        # Orthonormal QMF bank + untouched coefficients => inverse transform
        # is exactly the identity; reference's pad is a no-op for n = 2^16.
        denoised = x.copy()
    else:
        denoised = _dwt_backward_numpy(ds_full, a_full, scaling).astype(np.float32)

    return denoised, coeffs, res


def kernel(x, scaling):
    denoised, coeffs, _ = run(x, scaling)
    return denoised, coeffs


# revision 7
# speedup vs baseline: 1.1302x; 1.1302x over previous
"""Trainium2 Bass kernel for an 8-level circular DWT (forward + inverse).

The reference computes an 8-level periodized DWT (8-tap filters derived from
`scaling`) and returns (denoised, concat(coeffs)).  The inverse transform is
applied with no thresholding, so for orthonormal QMF filters (the DB4 bank
the reference ships) reconstruction is exactly the identity: denoised == x.
The kernel verifies that condition numerically and short-circuits the inverse
to a host-side copy; the forward transform runs on 8 NeuronCores,
data-parallel over rows.

Device math per level (length n per row, filters s/w, circular):
    d[j] = sum_k w[k] x[(2j-k) mod n],  a[j] = sum_k s[k] x[(2j-k) mod n]
with x laid out [p = seq mod 128] down partitions: X[p, r, b] = x_r[128b+p].
Both filters are packed into one pair of 128x128 banded stationaries per
output-column parity ("parity scheme"): output block c holds 64 a-outputs and
64 d-outputs (halves swap with c's parity so that the a-half always lands on
the partition range the next level's X layout needs):
    psum[:, c] = M_pi.T @ X[:, block c] + C_pi.T @ X[:, block c-1]
Approx halves are copied PSUM->SBUF partition-aligned (cast to f32r, which
the PE requires for its fast fp32 mode); detail halves stage through SBUF
and DMA out in a blocked layout the host untransposes.

Matmuls run in float32r (TF32-like, 1s/8e/11m, 2 PE cycles/row) with inputs
rounded host-side; coefficient L2 error vs the fp32 reference is ~2e-4.
"""

import sys
from contextlib import ExitStack

for _p in ("/opt/trn_rl_repo", "/root/.axon_site/_ro/trn_rl_repo"):
    if _p not in sys.path:
        sys.path.append(_p)

import numpy as np

import concourse.bacc as bacc
import concourse.mybir as mybir
import concourse.tile as tile
from concourse.bass_utils import run_bass_kernel_spmd

F32 = mybir.dt.float32
F32R = mybir.dt.float32r

N_ROWS = 512          # total rows
N0 = 65536            # row length (power of two: reference pad is a no-op)
LEVELS = 8
N_CORES = 8
ROWS = N_ROWS // N_CORES   # rows per core
RG_ROWS = 32               # rows per rowgroup (2 rowgroups pipelined)
SC_MAX = 4                 # d-out chunks batched per DMA


# ----------------------------- host-side math -----------------------------

def _wavelet(s):
    g = s[::-1].copy()
    sign = np.where(np.arange(s.shape[-1]) % 2 == 1, -1.0, 1.0).astype(g.dtype)
    return g * sign


def _make_parity_stationaries(s):
    """[M0, C0, M1, C1] (128,128) each, [p_in, m] layout (lhsT).

    m < 64 is the a-half for even output columns (parity 0) and the d-half
    for odd columns; m >= 64 the reverse.  M is the in-block band, C the
    wrap band reading the previous 128-input block.
    """
    w = _wavelet(s)
    mats = np.zeros((4, 128, 128), dtype=np.float32)
    for pi in (0, 1):
        M, C = mats[2 * pi], mats[2 * pi + 1]
        for m in range(128):
            a_out = (m < 64) == (pi == 0)
            q = m % 64
            g = s if a_out else w
            for k in range(8):
                p = 2 * q - k
                if p >= 0:
                    M[p, m] = g[k]
                else:
                    C[p + 128, m] = g[k]
    return mats


def _make_wmat(scaling):
    mats = []
    for lvl in range(LEVELS):
        s = np.asarray(scaling[lvl], dtype=np.float32)
        mats.append(_make_parity_stationaries(s))
    allw = np.concatenate(mats, axis=0)  # [LEVELS*4, 128, 128] (i, p, m)
    return np.ascontiguousarray(allw.transpose(1, 0, 2).reshape(128, -1))


def _round_f32r(arr):
    """Round fp32 to the nearest FP32R value (1s/8e/11m; low 12 bits zero)."""
    u = np.ascontiguousarray(arr, dtype=np.float32).view(np.uint32)
    r = (u + 0x7FF + ((u >> 12) & 1)) & np.uint32(0xFFFFF000)
    return r.view(np.float32)


def _pack_x_shard(x_rows):
    rows, n = x_rows.shape
    nb = n // 128
    blocks = x_rows.reshape(rows, nb, 128).transpose(2, 0, 1)  # [p, r, b]
    xt = np.empty((128, rows, nb + 1), dtype=np.float32)
    xt[:, :, 1:] = blocks
    xt[:, :, 0] = blocks[:, :, nb - 1]           # circular halo column
    return np.ascontiguousarray(xt.reshape(128, rows * (nb + 1)))


def _unpack_blocks(arr, rows):
    """[128, rows*nob] natural block layout -> [rows, nob*128]."""
    nob = arr.shape[1] // rows
    return arr.reshape(128, rows, nob).transpose(1, 2, 0).reshape(rows, nob * 128)


def _unpack_d_parity(arr, rows):
    """Parity-packed detail layout -> [rows, n/2].

    arr [128, rows*nbh]: partition 64+q col (r, cb) = d[r, 128cb + q]
    (even output column), partition q = d[r, 128cb + 64 + q] (odd column).
    """
    nbh = arr.shape[1] // rows
    a3 = arr.reshape(128, rows, nbh)
    out = np.empty((rows, nbh, 2, 64), dtype=arr.dtype)
    out[:, :, 0, :] = a3[64:128].transpose(1, 2, 0)
    out[:, :, 1, :] = a3[0:64].transpose(1, 2, 0)
    return out.reshape(rows, nbh * 128)


def _is_orthonormal_qmf(scaling):
    s = np.asarray(scaling, dtype=np.float64)
    if s.shape != (LEVELS, 8):
        return False
    for lvl in range(LEVELS):
        f = s[lvl]
        for m in range(4):
            v = np.dot(f[: 8 - 2 * m], f[2 * m:])
            if abs(v - (1.0 if m == 0 else 0.0)) > 1e-4:
                return False
    return True


def _dwt_backward_numpy(ds, a, scaling):
    """Fallback inverse transform (float64 FFT) for non-orthonormal filters."""
    a = np.asarray(a, dtype=np.float64)
    for lvl in reversed(range(LEVELS)):
        s = np.asarray(scaling[lvl], dtype=np.float64)
        w = _wavelet(s)
        d = np.asarray(ds[lvl], dtype=np.float64)
        n = d.shape[-1] * 2
        fd = np.zeros((d.shape[0], n))
        fd[:, ::2] = d
        fa = np.zeros((a.shape[0], n))
        fa[:, ::2] = a
        a = (np.fft.irfft(np.fft.rfft(fd, axis=-1)
                          * np.conj(np.fft.rfft(w, n=n)), n=n, axis=-1)
             + np.fft.irfft(np.fft.rfft(fa, axis=-1)
                            * np.conj(np.fft.rfft(s, n=n)), n=n, axis=-1))
    return a


# ----------------------------- device kernel ------------------------------

def _build_dwt(tc, xt, wmat, d_outs, a_out, n0=N0, rows=ROWS, levels=LEVELS,
               rg_rows=RG_ROWS):
    nc = tc.nc
    nb0 = n0 // 128
    n_rg = rows // rg_rows
    with ExitStack() as ctx:
        wpool = ctx.enter_context(tc.tile_pool(name="wpool", bufs=1))
        x0pool = ctx.enter_context(tc.tile_pool(name="x0pool", bufs=4))
        xpool = ctx.enter_context(tc.tile_pool(name="xpool", bufs=2))
        stpool = ctx.enter_context(tc.tile_pool(name="stpool", bufs=3))
        p0pool = ctx.enter_context(tc.tile_pool(name="p0pool", bufs=3, space="PSUM"))
        p1pool = ctx.enter_context(tc.tile_pool(name="p1pool", bufs=3, space="PSUM"))

        W = wpool.tile([128, levels * 4 * 128], F32R, name="Wsb")
        nc.sync.dma_start(W[:], wmat[:])

        xt3 = xt.rearrange("p (r b) -> p r b", b=nb0 + 1)
        Xs = {}  # (rg, lvl>=1) -> tile

        def do_level(rg, lvl):
            nb = (n0 >> lvl) // 128
            nbh = nb // 2
            nr = min(rg_rows, max(1, 512 // nbh))
            nchunks = rg_rows // nr
            sc = min(SC_MAX, nchunks)
            last = lvl + 1 == levels
            dh = d_outs[lvl].rearrange("p (r c) -> p r c", c=nbh)
            if last:
                ah = a_out.rearrange("p (r c) -> p r c", c=nbh)
            else:
                Xn = xpool.tile([128, rg_rows, nbh + 1], F32R,
                                name=f"X{lvl + 1}_{rg}", tag=f"X{lvl + 1}")
                Xs[(rg, lvl + 1)] = Xn
            k0 = lvl * 4 * 128
            M0, C0 = W[:, k0:k0 + 128], W[:, k0 + 128:k0 + 256]
            M1, C1 = W[:, k0 + 256:k0 + 384], W[:, k0 + 384:k0 + 512]

            st = sta = None
            for ch in range(nchunks):
                r0 = ch * nr            # row offset within rowgroup
                rg0 = rg * rg_rows + r0  # global row offset
                ws = slice(r0, r0 + nr)   # rowgroup-local rows (writes)
                if lvl == 0:
                    Xc = x0pool.tile([128, nr, nb + 1], F32R, tag="x0t", name="x0t")
                    nc.sync.dma_start(Xc[:], xt3[:, rg0:rg0 + nr, :])
                    rs = slice(0, nr)
                else:
                    Xc = Xs[(rg, lvl)]
                    rs = ws

                ps0 = p0pool.tile([128, nr, nbh], F32, tag="ps0", name="ps0")
                ps1 = p1pool.tile([128, nr, nbh], F32, tag="ps1", name="ps1")
                # X col 0 is the circular halo (last block); block b at col 1+b.
                # parity 0: main blocks 0,2,..; corner blocks -1(halo),1,3,..
                nc.tensor.matmul(ps0[:], M0, Xc[:, rs, 1:nb:2],
                                 start=True, stop=False)
                nc.tensor.matmul(ps0[:], C0, Xc[:, rs, 0:nb - 1:2],
                                 start=False, stop=True)
                # parity 1: main blocks 1,3,..; corner blocks 0,2,..
                nc.tensor.matmul(ps1[:], M1, Xc[:, rs, 2:nb + 1:2],
                                 start=True, stop=False)
                nc.tensor.matmul(ps1[:], C1, Xc[:, rs, 1:nb:2],
                                 start=False, stop=True)

                sci = ch % sc
                ss = slice(sci * nr, (sci + 1) * nr)
                if sci == 0:
                    st = stpool.tile([128, sc * nr, nbh], F32, tag="st",
                                     name="st")
                    if last:
                        sta = stpool.tile([128, sc * nr, nbh], F32, tag="sta",
                                          name="sta")
                # approx halves -> next level X (partition-aligned), f32r cast
                if not last:
                    nc.vector.tensor_copy(Xn[0:64, ws, 1:1 + nbh], ps0[0:64, :, :])
                    nc.scalar.copy(Xn[64:128, ws, 1:1 + nbh], ps1[64:128, :, :])
                    nc.vector.tensor_copy(Xn[0:64, ws, 0:1],
                                          ps0[0:64, :, nbh - 1:nbh])
                    nc.scalar.copy(Xn[64:128, ws, 0:1],
                                   ps1[64:128, :, nbh - 1:nbh])
                else:
                    nc.vector.tensor_copy(sta[0:64, ss, :], ps0[0:64, :, :])
                    nc.scalar.copy(sta[64:128, ss, :], ps1[64:128, :, :])
                # detail halves -> stage
                nc.vector.tensor_copy(st[0:64, ss, :], ps1[0:64, :, :])
                nc.scalar.copy(st[64:128, ss, :], ps0[64:128, :, :])

                if sci == sc - 1:
                    g0 = rg * rg_rows + (ch - sci) * nr
                    nc.scalar.dma_start(dh[:, g0:g0 + sc * nr, :], st[:])
                    if last:
                        nc.scalar.dma_start(ah[:, g0:g0 + sc * nr, :], sta[:])

        # wavefront order: rowgroup 1's level l interleaves with rowgroup 0's
        # level l+1 so the PE always has independent work across the level
        # dependency chain (and rowgroup 1's input DMA hides under rowgroup
        # 0's compute).
        order = []
        for step in range(levels + n_rg - 1):
            for rg in range(n_rg):
                lvl = step - rg
                if 0 <= lvl < levels:
                    order.append((rg, lvl))
        for rg, lvl in order:
            do_level(rg, lvl)


_MODULE_CACHE = {}


def _get_module():
    if "nc" in _MODULE_CACHE:
        return _MODULE_CACHE["nc"]
    nc = bacc.Bacc("TRN2", target_bir_lowering=False, debug=False,
                   num_devices=N_CORES)
    xt = nc.dram_tensor("xt", [128, ROWS * (N0 // 128 + 1)], F32R,
                        kind="ExternalInput").ap()
    wmat = nc.dram_tensor("wmat", [128, LEVELS * 4 * 128], F32R,
                          kind="ExternalInput").ap()
    d_outs = []
    for lvl in range(LEVELS):
        nbh = (N0 >> lvl) // 256
        d_outs.append(nc.dram_tensor(f"d{lvl}", [128, ROWS * nbh], F32,
                                     kind="ExternalOutput").ap())
    a_out = nc.dram_tensor("aF", [128, ROWS * ((N0 >> (LEVELS - 1)) // 256)],
                           F32, kind="ExternalOutput").ap()
    with tile.TileContext(nc) as tc:
        _build_dwt(tc, xt, wmat, d_outs, a_out)
    nc.compile()
    _MODULE_CACHE["nc"] = nc
    return nc


def run(x, scaling, **spmd_kwargs):
    """Full pipeline.  Returns (denoised, coeffs, BassKernelResults)."""
    x = np.ascontiguousarray(np.asarray(x, dtype=np.float32))
    scaling = np.asarray(scaling, dtype=np.float32)
    assert x.shape == (N_ROWS, N0), x.shape
    assert scaling.shape == (LEVELS, 8), scaling.shape

    nc = _get_module()
    wmat = _round_f32r(_make_wmat(scaling))
    in_maps = []
    for c in range(N_CORES):
        in_maps.append({
            "xt": _pack_x_shard(_round_f32r(x[c * ROWS:(c + 1) * ROWS])),
            "wmat": wmat,
        })

    res = run_bass_kernel_spmd(nc, in_maps, core_ids=list(range(N_CORES)),
                               **spmd_kwargs)

    coeffs = np.empty((N_ROWS, N0), dtype=np.float32)
    off = 0
    ds_full = []
    for lvl in range(LEVELS):
        half = (N0 >> lvl) // 2
        dcols = coeffs[:, off:off + half]
        for c in range(N_CORES):
            dcols[c * ROWS:(c + 1) * ROWS] = _unpack_d_parity(
                res.results[c][f"d{lvl}"], ROWS)
        ds_full.append(dcols)
        off += half
    a_full = np.empty((N_ROWS, N0 - off), dtype=np.float32)
    for c in range(N_CORES):
        a_full[c * ROWS:(c + 1) * ROWS] = _unpack_blocks(
            res.results[c]["aF"], ROWS)
    coeffs[:, off:] = a_full

    if _is_orthonormal_qmf(scaling):
        # Orthonormal QMF bank + untouched coefficients => the inverse
        # transform is exactly the identity (reference pad is a no-op).
        denoised = x.copy()
    else:
        denoised = _dwt_backward_numpy(ds_full, a_full, scaling).astype(np.float32)

    return denoised, coeffs, res


def kernel(x, scaling):
    denoised, coeffs, _ = run(x, scaling)
    return denoised, coeffs


# revision 8
# speedup vs baseline: 1.1675x; 1.0330x over previous
"""Trainium2 Bass kernel for an 8-level circular DWT (forward + inverse).

The reference computes an 8-level periodized DWT (8-tap filters derived from
`scaling`) and returns (denoised, concat(coeffs)).  The inverse transform is
applied with no thresholding, so for orthonormal QMF filters (the DB4 bank
the reference ships) reconstruction is exactly the identity: denoised == x.
The kernel verifies that condition numerically and short-circuits the inverse
to a host-side copy; the forward transform runs on 8 NeuronCores,
data-parallel over rows.

Device math per level (length n per row, filters s/w, circular):
    d[j] = sum_k w[k] x[(2j-k) mod n],  a[j] = sum_k s[k] x[(2j-k) mod n]
with x laid out [p = seq mod 128] down partitions: X[p, r, b] = x_r[128b+p].
Both filters are packed into one pair of 128x128 banded stationaries per
output-column parity ("parity scheme"): output block c holds 64 a-outputs and
64 d-outputs (halves swap with c's parity so that the a-half always lands on
the partition range the next level's X layout needs):
    psum[:, c] = M_pi.T @ X[:, block c] + C_pi.T @ X[:, block c-1]
Approx halves are copied PSUM->SBUF partition-aligned (cast to f32r, which
the PE requires for its fast fp32 mode); detail halves stage through SBUF
and DMA out in a blocked layout the host untransposes.

Matmuls run in float32r (TF32-like, 1s/8e/11m, 2 PE cycles/row) with inputs
rounded host-side; coefficient L2 error vs the fp32 reference is ~2e-4.
"""

import sys
from contextlib import ExitStack

for _p in ("/opt/trn_rl_repo", "/root/.axon_site/_ro/trn_rl_repo"):
    if _p not in sys.path:
        sys.path.append(_p)

import numpy as np

import concourse.bacc as bacc
import concourse.mybir as mybir
import concourse.tile as tile
from concourse.bass_utils import run_bass_kernel_spmd

F32 = mybir.dt.float32
F32R = mybir.dt.float32r

N_ROWS = 512          # total rows
N0 = 65536            # row length (power of two: reference pad is a no-op)
LEVELS = 8
N_CORES = 8
ROWS = N_ROWS // N_CORES   # rows per core
RG_ROWS = 32               # rows per rowgroup (2 rowgroups pipelined)
SC_MAX = 4                 # d-out chunks batched per DMA


# ----------------------------- host-side math -----------------------------

def _wavelet(s):
    g = s[::-1].copy()
    sign = np.where(np.arange(s.shape[-1]) % 2 == 1, -1.0, 1.0).astype(g.dtype)
    return g * sign


def _make_parity_stationaries(s):
    """[M0, C0, M1, C1] (128,128) each, [p_in, m] layout (lhsT).

    m < 64 is the a-half for even output columns (parity 0) and the d-half
    for odd columns; m >= 64 the reverse.  M is the in-block band, C the
    wrap band reading the previous 128-input block.
    """
    w = _wavelet(s)
    mats = np.zeros((4, 128, 128), dtype=np.float32)
    for pi in (0, 1):
        M, C = mats[2 * pi], mats[2 * pi + 1]
        for m in range(128):
            a_out = (m < 64) == (pi == 0)
            q = m % 64
            g = s if a_out else w
            for k in range(8):
                p = 2 * q - k
                if p >= 0:
                    M[p, m] = g[k]
                else:
                    C[p + 128, m] = g[k]
    return mats


def _make_wmat(scaling):
    mats = []
    for lvl in range(LEVELS):
        s = np.asarray(scaling[lvl], dtype=np.float32)
        mats.append(_make_parity_stationaries(s))
    allw = np.concatenate(mats, axis=0)  # [LEVELS*4, 128, 128] (i, p, m)
    return np.ascontiguousarray(allw.transpose(1, 0, 2).reshape(128, -1))


def _round_f32r(arr):
    """Round fp32 to the nearest FP32R value (1s/8e/11m; low 12 bits zero)."""
    u = np.ascontiguousarray(arr, dtype=np.float32).view(np.uint32)
    r = (u + 0x7FF + ((u >> 12) & 1)) & np.uint32(0xFFFFF000)
    return r.view(np.float32)


def _pack_x_shard(x_rows):
    rows, n = x_rows.shape
    nb = n // 128
    blocks = x_rows.reshape(rows, nb, 128).transpose(2, 0, 1)  # [p, r, b]
    xt = np.empty((128, rows, nb + 1), dtype=np.float32)
    xt[:, :, 1:] = blocks
    xt[:, :, 0] = blocks[:, :, nb - 1]           # circular halo column
    return np.ascontiguousarray(xt.reshape(128, rows * (nb + 1)))


def _unpack_blocks(arr, rows):
    """[128, rows*nob] natural block layout -> [rows, nob*128]."""
    nob = arr.shape[1] // rows
    return arr.reshape(128, rows, nob).transpose(1, 2, 0).reshape(rows, nob * 128)


def _unpack_d_parity(arr, rows):
    """Parity-packed detail layout -> [rows, n/2].

    arr [128, rows*nbh]: partition 64+q col (r, cb) = d[r, 128cb + q]
    (even output column), partition q = d[r, 128cb + 64 + q] (odd column).
    """
    nbh = arr.shape[1] // rows
    a3 = arr.reshape(128, rows, nbh)
    out = np.empty((rows, nbh, 2, 64), dtype=arr.dtype)
    out[:, :, 0, :] = a3[64:128].transpose(1, 2, 0)
    out[:, :, 1, :] = a3[0:64].transpose(1, 2, 0)
    return out.reshape(rows, nbh * 128)


def _is_orthonormal_qmf(scaling):
    s = np.asarray(scaling, dtype=np.float64)
    if s.shape != (LEVELS, 8):
        return False
    for lvl in range(LEVELS):
        f = s[lvl]
        for m in range(4):
            v = np.dot(f[: 8 - 2 * m], f[2 * m:])
            if abs(v - (1.0 if m == 0 else 0.0)) > 1e-4:
                return False
    return True


def _dwt_backward_numpy(ds, a, scaling):
    """Fallback inverse transform (float64 FFT) for non-orthonormal filters."""
    a = np.asarray(a, dtype=np.float64)
    for lvl in reversed(range(LEVELS)):
        s = np.asarray(scaling[lvl], dtype=np.float64)
        w = _wavelet(s)
        d = np.asarray(ds[lvl], dtype=np.float64)
        n = d.shape[-1] * 2
        fd = np.zeros((d.shape[0], n))
        fd[:, ::2] = d
        fa = np.zeros((a.shape[0], n))
        fa[:, ::2] = a
        a = (np.fft.irfft(np.fft.rfft(fd, axis=-1)
                          * np.conj(np.fft.rfft(w, n=n)), n=n, axis=-1)
             + np.fft.irfft(np.fft.rfft(fa, axis=-1)
                            * np.conj(np.fft.rfft(s, n=n)), n=n, axis=-1))
    return a


# ----------------------------- device kernel ------------------------------

def _build_dwt(tc, xt, wmat, d_outs, a_out, n0=N0, rows=ROWS, levels=LEVELS,
               rg_rows=RG_ROWS):
    nc = tc.nc
    nb0 = n0 // 128
    n_rg = rows // rg_rows
    with ExitStack() as ctx:
        wpool = ctx.enter_context(tc.tile_pool(name="wpool", bufs=1))
        x0pool = ctx.enter_context(tc.tile_pool(name="x0pool", bufs=3))
        xpool = ctx.enter_context(tc.tile_pool(name="xpool", bufs=2))
        stpool = ctx.enter_context(tc.tile_pool(name="stpool", bufs=2))
        p0pool = ctx.enter_context(tc.tile_pool(name="p0pool", bufs=3, space="PSUM"))
        p1pool = ctx.enter_context(tc.tile_pool(name="p1pool", bufs=3, space="PSUM"))

        W = wpool.tile([128, levels * 4 * 128], F32R, name="Wsb")
        w_loaded = set()

        def load_w(lvl):
            if lvl in w_loaded:
                return
            w_loaded.add(lvl)
            k0 = lvl * 4 * 128
            nc.sync.dma_start(W[:, k0:k0 + 512], wmat[:, k0:k0 + 512])

        xt3 = xt.rearrange("p (r b) -> p r b", b=nb0 + 1)
        Xs = {}  # (rg, lvl>=1) -> tile

        def do_level(rg, lvl):
            nb = (n0 >> lvl) // 128
            nbh = nb // 2
            nr = min(rg_rows, max(1, 512 // nbh))
            nchunks = rg_rows // nr
            sc = min(SC_MAX, nchunks)
            last = lvl + 1 == levels
            dh = d_outs[lvl].rearrange("p (r c) -> p r c", c=nbh)
            if last:
                ah = a_out.rearrange("p (r c) -> p r c", c=nbh)
            else:
                Xn = xpool.tile([128, rg_rows, nbh + 1], F32R,
                                name=f"X{lvl + 1}_{rg}", tag=f"X{lvl + 1}")
                Xs[(rg, lvl + 1)] = Xn
            load_w(lvl)
            k0 = lvl * 4 * 128
            M0, C0 = W[:, k0:k0 + 128], W[:, k0 + 128:k0 + 256]
            M1, C1 = W[:, k0 + 256:k0 + 384], W[:, k0 + 384:k0 + 512]

            st = sta = None
            for ch in range(nchunks):
                r0 = ch * nr            # row offset within rowgroup
                rg0 = rg * rg_rows + r0  # global row offset
                ws = slice(r0, r0 + nr)   # rowgroup-local rows (writes)
                if lvl == 0:
                    if ch % 2 == 0:
                        nld = min(2 * nr, rg_rows - r0)
                        x0t = x0pool.tile([128, 2 * nr, nb + 1], F32R,
                                          tag="x0t", name="x0t")
                        nc.sync.dma_start(x0t[:, 0:nld, :],
                                          xt3[:, rg0:rg0 + nld, :])
                    Xc = x0t
                    rs = slice((ch % 2) * nr, (ch % 2) * nr + nr)
                else:
                    Xc = Xs[(rg, lvl)]
                    rs = ws

                ps0 = p0pool.tile([128, nr, nbh], F32, tag="ps0", name="ps0")
                ps1 = p1pool.tile([128, nr, nbh], F32, tag="ps1", name="ps1")
                # X col 0 is the circular halo (last block); block b at col 1+b.
                # parity 0: main blocks 0,2,..; corner blocks -1(halo),1,3,..
                nc.tensor.matmul(ps0[:], M0, Xc[:, rs, 1:nb:2],
                                 start=True, stop=False)
                # parity 1: main blocks 1,3,..; corner blocks 0,2,..
                nc.tensor.matmul(ps1[:], M1, Xc[:, rs, 2:nb + 1:2],
                                 start=True, stop=False)
                nc.tensor.matmul(ps1[:], C1, Xc[:, rs, 1:nb:2],
                                 start=False, stop=True)
                nc.tensor.matmul(ps0[:], C0, Xc[:, rs, 0:nb - 1:2],
                                 start=False, stop=True)

                sci = ch % sc
                ss = slice(sci * nr, (sci + 1) * nr)
                if sci == 0:
                    st = stpool.tile([128, sc * nr, nbh], F32, tag="st",
                                     name="st")
                    if last:
                        sta = stpool.tile([128, sc * nr, nbh], F32, tag="sta",
                                          name="sta")
                # approx halves -> next level X (partition-aligned), f32r cast
                if not last:
                    nc.vector.tensor_copy(Xn[0:64, ws, 1:1 + nbh], ps0[0:64, :, :])
                    nc.scalar.copy(Xn[64:128, ws, 1:1 + nbh], ps1[64:128, :, :])
                else:
                    nc.vector.tensor_copy(sta[0:64, ss, :], ps0[0:64, :, :])
                    nc.scalar.copy(sta[64:128, ss, :], ps1[64:128, :, :])
                # detail halves -> stage
                nc.vector.tensor_copy(st[0:64, ss, :], ps1[0:64, :, :])
                nc.scalar.copy(st[64:128, ss, :], ps0[64:128, :, :])

                if sci == sc - 1:
                    g0 = rg * rg_rows + (ch - sci) * nr
                    nc.scalar.dma_start(dh[:, g0:g0 + sc * nr, :], st[:])
                    if last:
                        nc.scalar.dma_start(ah[:, g0:g0 + sc * nr, :], sta[:])
            if not last:
                # circular halo column for the whole rowgroup in one op
                nc.vector.tensor_copy(Xn[:, :, 0:1], Xn[:, :, nbh:nbh + 1])

        # wavefront order: rowgroup 1's level l interleaves with rowgroup 0's
        # level l+1 so the PE always has independent work across the level
        # dependency chain (and rowgroup 1's input DMA hides under rowgroup
        # 0's compute).
        order = []
        for step in range(levels + n_rg - 1):
            for rg in range(n_rg):
                lvl = step - rg
                if 0 <= lvl < levels:
                    order.append((rg, lvl))
        for rg, lvl in order:
            do_level(rg, lvl)


_MODULE_CACHE = {}


def _get_module():
    if "nc" in _MODULE_CACHE:
        return _MODULE_CACHE["nc"]
    nc = bacc.Bacc("TRN2", target_bir_lowering=False, debug=False,
                   num_devices=N_CORES)
    xt = nc.dram_tensor("xt", [128, ROWS * (N0 // 128 + 1)], F32R,
                        kind="ExternalInput").ap()
    wmat = nc.dram_tensor("wmat", [128, LEVELS * 4 * 128], F32R,
                          kind="ExternalInput").ap()
    d_outs = []
    for lvl in range(LEVELS):
        nbh = (N0 >> lvl) // 256
        d_outs.append(nc.dram_tensor(f"d{lvl}", [128, ROWS * nbh], F32,
                                     kind="ExternalOutput").ap())
    a_out = nc.dram_tensor("aF", [128, ROWS * ((N0 >> (LEVELS - 1)) // 256)],
                           F32, kind="ExternalOutput").ap()
    with tile.TileContext(nc) as tc:
        _build_dwt(tc, xt, wmat, d_outs, a_out)
    nc.compile()
    _MODULE_CACHE["nc"] = nc
    return nc


def run(x, scaling, **spmd_kwargs):
    """Full pipeline.  Returns (denoised, coeffs, BassKernelResults)."""
    x = np.ascontiguousarray(np.asarray(x, dtype=np.float32))
    scaling = np.asarray(scaling, dtype=np.float32)
    assert x.shape == (N_ROWS, N0), x.shape
    assert scaling.shape == (LEVELS, 8), scaling.shape

    nc = _get_module()
    wmat = _round_f32r(_make_wmat(scaling))
    in_maps = []
    for c in range(N_CORES):
        in_maps.append({
            "xt": _pack_x_shard(_round_f32r(x[c * ROWS:(c + 1) * ROWS])),
            "wmat": wmat,
        })

    res = run_bass_kernel_spmd(nc, in_maps, core_ids=list(range(N_CORES)),
                               **spmd_kwargs)

    coeffs = np.empty((N_ROWS, N0), dtype=np.float32)
    off = 0
    ds_full = []
    for lvl in range(LEVELS):
        half = (N0 >> lvl) // 2
        dcols = coeffs[:, off:off + half]
        for c in range(N_CORES):
            dcols[c * ROWS:(c + 1) * ROWS] = _unpack_d_parity(
                res.results[c][f"d{lvl}"], ROWS)
        ds_full.append(dcols)
        off += half
    a_full = np.empty((N_ROWS, N0 - off), dtype=np.float32)
    for c in range(N_CORES):
        a_full[c * ROWS:(c + 1) * ROWS] = _unpack_blocks(
            res.results[c]["aF"], ROWS)
    coeffs[:, off:] = a_full

    if _is_orthonormal_qmf(scaling):
        # Orthonormal QMF bank + untouched coefficients => the inverse
        # transform is exactly the identity (reference pad is a no-op).
        denoised = x.copy()
    else:
        denoised = _dwt_backward_numpy(ds_full, a_full, scaling).astype(np.float32)

    return denoised, coeffs, res


def kernel(x, scaling):
    denoised, coeffs, _ = run(x, scaling)
    return denoised, coeffs


# revision 10
# speedup vs baseline: 1.2922x; 1.1068x over previous
"""Trainium2 Bass kernel for an 8-level circular DWT (forward + inverse).

The reference computes an 8-level periodized DWT (8-tap filters derived from
`scaling`) and returns (denoised, concat(coeffs)).  The inverse transform is
applied with no thresholding, so for orthonormal QMF filters (the DB4 bank
the reference ships) reconstruction is exactly the identity: denoised == x.
The kernel verifies that condition numerically and short-circuits the inverse
to a host-side copy; the forward transform runs on 8 NeuronCores,
data-parallel over rows.

Device math per level (length n per row, filters s/w, circular):
    d[j] = sum_k w[k] x[(2j-k) mod n],  a[j] = sum_k s[k] x[(2j-k) mod n]
with x laid out [p = seq mod 128] down partitions: X[p, r, b] = x_r[128b+p].
Both filters are packed into one pair of 128x128 banded stationaries per
output-column parity ("parity scheme"): output block c holds 64 a-outputs and
64 d-outputs (halves swap with c's parity so that the a-half always lands on
the partition range the next level's X layout needs):
    psum[:, c] = M_pi.T @ X[:, block c] + C_pi.T @ X[:, block c-1]
Approx halves are copied PSUM->SBUF partition-aligned (cast to f32r, which
the PE requires for its fast fp32 mode); detail halves stage through SBUF
and DMA out in a blocked layout the host untransposes.

Matmuls run in float32r (TF32-like, 1s/8e/11m, 2 PE cycles/row) with inputs
rounded host-side; coefficient L2 error vs the fp32 reference is ~2e-4.
"""

import sys
from contextlib import ExitStack

for _p in ("/opt/trn_rl_repo", "/root/.axon_site/_ro/trn_rl_repo"):
    if _p not in sys.path:
        sys.path.append(_p)

import numpy as np

import concourse.bacc as bacc
import concourse.mybir as mybir
import concourse.tile as tile
from concourse.bass_utils import run_bass_kernel_spmd

F32 = mybir.dt.float32
F32R = mybir.dt.float32r

N_ROWS = 512          # total rows
N0 = 65536            # row length (power of two: reference pad is a no-op)
LEVELS = 8
N_CORES = 8
ROWS = N_ROWS // N_CORES   # rows per core
RG_ROWS = 16               # rows per rowgroup for levels 0-2
SC_MAX = 4                 # d-out chunks batched per DMA


# ----------------------------- host-side math -----------------------------

def _wavelet(s):
    g = s[::-1].copy()
    sign = np.where(np.arange(s.shape[-1]) % 2 == 1, -1.0, 1.0).astype(g.dtype)
    return g * sign


def _make_parity_stationaries(s):
    """[M0, C0, M1, C1] (128,128) each, [p_in, m] layout (lhsT).

    m < 64 is the a-half for even output columns (parity 0) and the d-half
    for odd columns; m >= 64 the reverse.  M is the in-block band, C the
    wrap band reading the previous 128-input block.
    """
    w = _wavelet(s)
    mats = np.zeros((4, 128, 128), dtype=np.float32)
    for pi in (0, 1):
        M, C = mats[2 * pi], mats[2 * pi + 1]
        for m in range(128):
            a_out = (m < 64) == (pi == 0)
            q = m % 64
            g = s if a_out else w
            for k in range(8):
                p = 2 * q - k
                if p >= 0:
                    M[p, m] = g[k]
                else:
                    C[p + 128, m] = g[k]
    return mats


def _make_wmat(scaling):
    mats = []
    for lvl in range(LEVELS):
        s = np.asarray(scaling[lvl], dtype=np.float32)
        mats.append(_make_parity_stationaries(s))
    allw = np.concatenate(mats, axis=0)  # [LEVELS*4, 128, 128] (i, p, m)
    return np.ascontiguousarray(allw.transpose(1, 0, 2).reshape(128, -1))


def _round_f32r(arr):
    """Round fp32 to the nearest FP32R value (1s/8e/11m; low 12 bits zero)."""
    u = np.ascontiguousarray(arr, dtype=np.float32).view(np.uint32)
    r = (u + 0x7FF + ((u >> 12) & 1)) & np.uint32(0xFFFFF000)
    return r.view(np.float32)


def _pack_x_shard(x_rows):
    rows, n = x_rows.shape
    nb = n // 128
    blocks = x_rows.reshape(rows, nb, 128).transpose(2, 0, 1)  # [p, r, b]
    xt = np.empty((128, rows, nb + 1), dtype=np.float32)
    xt[:, :, 1:] = blocks
    xt[:, :, 0] = blocks[:, :, nb - 1]           # circular halo column
    return np.ascontiguousarray(xt.reshape(128, rows * (nb + 1)))


def _unpack_blocks(arr, rows):
    """[128, rows*nob] natural block layout -> [rows, nob*128]."""
    nob = arr.shape[1] // rows
    return arr.reshape(128, rows, nob).transpose(1, 2, 0).reshape(rows, nob * 128)


def _unpack_d_parity(arr, rows):
    """Parity-packed detail layout -> [rows, n/2].

    arr [128, rows*nbh]: partition 64+q col (r, cb) = d[r, 128cb + q]
    (even output column), partition q = d[r, 128cb + 64 + q] (odd column).
    """
    nbh = arr.shape[1] // rows
    a3 = arr.reshape(128, rows, nbh)
    out = np.empty((rows, nbh, 2, 64), dtype=arr.dtype)
    out[:, :, 0, :] = a3[64:128].transpose(1, 2, 0)
    out[:, :, 1, :] = a3[0:64].transpose(1, 2, 0)
    return out.reshape(rows, nbh * 128)


def _is_orthonormal_qmf(scaling):
    s = np.asarray(scaling, dtype=np.float64)
    if s.shape != (LEVELS, 8):
        return False
    for lvl in range(LEVELS):
        f = s[lvl]
        for m in range(4):
            v = np.dot(f[: 8 - 2 * m], f[2 * m:])
            if abs(v - (1.0 if m == 0 else 0.0)) > 1e-4:
                return False
    return True


def _dwt_backward_numpy(ds, a, scaling):
    """Fallback inverse transform (float64 FFT) for non-orthonormal filters."""
    a = np.asarray(a, dtype=np.float64)
    for lvl in reversed(range(LEVELS)):
        s = np.asarray(scaling[lvl], dtype=np.float64)
        w = _wavelet(s)
        d = np.asarray(ds[lvl], dtype=np.float64)
        n = d.shape[-1] * 2
        fd = np.zeros((d.shape[0], n))
        fd[:, ::2] = d
        fa = np.zeros((a.shape[0], n))
        fa[:, ::2] = a
        a = (np.fft.irfft(np.fft.rfft(fd, axis=-1)
                          * np.conj(np.fft.rfft(w, n=n)), n=n, axis=-1)
             + np.fft.irfft(np.fft.rfft(fa, axis=-1)
                            * np.conj(np.fft.rfft(s, n=n)), n=n, axis=-1))
    return a


# ----------------------------- device kernel ------------------------------

def _build_dwt(tc, xt, wmat, d_outs, a_out, n0=N0, rows=ROWS, levels=LEVELS,
               rg_rows=RG_ROWS):
    """Forward DWT, parity scheme.

    Levels 0..2 are processed in `rows/rg_rows` independent row-groups,
    wavefront-interleaved so the level-0 input streaming (the bulk of HBM
    traffic) spreads across most of the kernel instead of bunching at the
    start; levels 3+ run once over all rows (keeps matmul free dims large).
    """
    nc = tc.nc
    nb0 = n0 // 128
    n_rg = rows // rg_rows
    FINE_LVLS = 3  # levels below this are row-grouped
    with ExitStack() as ctx:
        wpool = ctx.enter_context(tc.tile_pool(name="wpool", bufs=1))
        x0pool = ctx.enter_context(tc.tile_pool(name="x0pool", bufs=6))
        xpool = ctx.enter_context(tc.tile_pool(name="xpool", bufs=2))
        x1pool = ctx.enter_context(tc.tile_pool(name="x1pool", bufs=1))
        stpool = ctx.enter_context(tc.tile_pool(name="stpool", bufs=2))
        p0pool = ctx.enter_context(tc.tile_pool(name="p0pool", bufs=3, space="PSUM"))
        p1pool = ctx.enter_context(tc.tile_pool(name="p1pool", bufs=3, space="PSUM"))

        W = wpool.tile([128, levels * 4 * 128], F32R, name="Wsb")
        w_loaded = set()

        def load_w(lvl):
            if lvl in w_loaded:
                return
            w_loaded.add(lvl)
            k0 = lvl * 4 * 128
            nc.sync.dma_start(W[:, k0:k0 + 512], wmat[:, k0:k0 + 512])

        xt3 = xt.rearrange("p (r b) -> p r b", b=nb0 + 1)
        Xs = {}          # (key, lvl) -> SBUF X tile; key = rg or "all"
        halo_done = set()

        def do_unit(rg, lvl):
            """Process rows [row0, row0+nrows) of level lvl."""
            fine = lvl < FINE_LVLS
            row0 = rg * rg_rows if fine else 0
            nrows = rg_rows if fine else rows
            nb = (n0 >> lvl) // 128
            nbh = nb // 2
            nr = min(nrows, max(1, 512 // nbh))
            nchunks = nrows // nr
            sc = min(SC_MAX, nchunks)
            last = lvl + 1 == levels
            load_w(lvl)
            dh = d_outs[lvl].rearrange("p (r c) -> p r c", c=nbh)
            if last:
                ah = a_out.rearrange("p (r c) -> p r c", c=nbh)
            else:
                nfine = lvl + 1 < FINE_LVLS
                nkey = (rg if nfine else "all", lvl + 1)
                if nkey not in Xs:
                    xrows = rg_rows if nfine else rows
                    pool = xpool if nfine else x1pool
                    Xs[nkey] = pool.tile([128, xrows, nbh + 1], F32R,
                                         name=f"X{lvl + 1}_{nkey[0]}",
                                         tag=f"X{lvl + 1}")
                Xn = Xs[nkey]

            if lvl > 0:
                key = (rg, lvl) if fine else ("all", lvl)
                Xl = Xs[key]
                if key not in halo_done:
                    halo_done.add(key)
                    # circular halo: col 0 := last block (col nb)
                    nc.vector.tensor_copy(Xl[:, :, 0:1], Xl[:, :, nb:nb + 1])

            k0 = lvl * 4 * 128
            M0, C0 = W[:, k0:k0 + 128], W[:, k0 + 128:k0 + 256]
            M1, C1 = W[:, k0 + 256:k0 + 384], W[:, k0 + 384:k0 + 512]

            st = sta = None
            for ch in range(nchunks):
                r0 = ch * nr               # row offset within this unit
                g0 = row0 + r0             # global row offset
                if lvl == 0:
                    Xc = x0pool.tile([128, nr, nb + 1], F32R, tag="x0t",
                                     name="x0t")
                    nc.sync.dma_start(Xc[:], xt3[:, g0:g0 + nr, :])
                    rs = slice(0, nr)
                else:
                    # fine-level X tiles are rowgroup-local; coarse have r0==g0
                    Xc = Xl
                    rs = slice(r0, r0 + nr)

                ps0 = p0pool.tile([128, nr, nbh], F32, tag="ps0", name="ps0")
                ps1 = p1pool.tile([128, nr, nbh], F32, tag="ps1", name="ps1")
                # X col 0 = circular halo; block b at col 1+b.
                # parity 0: main blocks 0,2,..; corner blocks -1(halo),1,3,..
                # parity 1: main blocks 1,3,..; corner blocks 0,2,..
                nc.tensor.matmul(ps0[:], M0, Xc[:, rs, 1:nb:2],
                                 start=True, stop=False)
                nc.tensor.matmul(ps1[:], M1, Xc[:, rs, 2:nb + 1:2],
                                 start=True, stop=False)
                nc.tensor.matmul(ps1[:], C1, Xc[:, rs, 1:nb:2],
                                 start=False, stop=True)
                nc.tensor.matmul(ps0[:], C0, Xc[:, rs, 0:nb - 1:2],
                                 start=False, stop=True)

                sci = ch % sc
                ss = slice(sci * nr, (sci + 1) * nr)
                if sci == 0:
                    st = stpool.tile([128, sc * nr, nbh], F32, tag="st",
                                     name="st")
                    if last:
                        sta = stpool.tile([128, sc * nr, nbh], F32, tag="sta",
                                          name="sta")
                # approx halves -> next level X (partition-aligned, f32r cast)
                if not last:
                    wr = (slice(r0, r0 + nr) if lvl + 1 < FINE_LVLS
                          else slice(g0, g0 + nr))
                    nc.vector.tensor_copy(Xn[0:64, wr, 1:1 + nbh],
                                          ps0[0:64, :, :])
                    nc.scalar.copy(Xn[64:128, wr, 1:1 + nbh],
                                   ps1[64:128, :, :])
                else:
                    nc.vector.tensor_copy(sta[0:64, ss, :], ps0[0:64, :, :])
                    nc.scalar.copy(sta[64:128, ss, :], ps1[64:128, :, :])
                # detail halves -> stage
                nc.vector.tensor_copy(st[0:64, ss, :], ps1[0:64, :, :])
                nc.scalar.copy(st[64:128, ss, :], ps0[64:128, :, :])

                if sci == sc - 1:
                    d0 = row0 + (ch - sci) * nr
                    nc.scalar.dma_start(dh[:, d0:d0 + sc * nr, :], st[:])
                    if last:
                        nc.scalar.dma_start(ah[:, d0:d0 + sc * nr, :], sta[:])

        # wavefront over (rg, lvl) for fine levels, then coarse levels
        order = []
        for step in range(FINE_LVLS + n_rg - 1):
            for rg in range(n_rg):
                lvl = step - rg
                if 0 <= lvl < FINE_LVLS:
                    order.append((rg, lvl))
        for lvl in range(FINE_LVLS, levels):
            order.append((0, lvl))
        for rg, lvl in order:
            do_unit(rg, lvl)


_MODULE_CACHE = {}


def _get_module():
    if "nc" in _MODULE_CACHE:
        return _MODULE_CACHE["nc"]
    nc = bacc.Bacc("TRN2", target_bir_lowering=False, debug=False,
                   num_devices=N_CORES)
    xt = nc.dram_tensor("xt", [128, ROWS * (N0 // 128 + 1)], F32R,
                        kind="ExternalInput").ap()
    wmat = nc.dram_tensor("wmat", [128, LEVELS * 4 * 128], F32R,
                          kind="ExternalInput").ap()
    d_outs = []
    for lvl in range(LEVELS):
        nbh = (N0 >> lvl) // 256
        d_outs.append(nc.dram_tensor(f"d{lvl}", [128, ROWS * nbh], F32,
                                     kind="ExternalOutput").ap())
    a_out = nc.dram_tensor("aF", [128, ROWS * ((N0 >> (LEVELS - 1)) // 256)],
                           F32, kind="ExternalOutput").ap()
    with tile.TileContext(nc) as tc:
        _build_dwt(tc, xt, wmat, d_outs, a_out)
    nc.compile()
    _MODULE_CACHE["nc"] = nc
    return nc


def run(x, scaling, **spmd_kwargs):
    """Full pipeline.  Returns (denoised, coeffs, BassKernelResults)."""
    x = np.ascontiguousarray(np.asarray(x, dtype=np.float32))
    scaling = np.asarray(scaling, dtype=np.float32)
    assert x.shape == (N_ROWS, N0), x.shape
    assert scaling.shape == (LEVELS, 8), scaling.shape

    nc = _get_module()
    wmat = _round_f32r(_make_wmat(scaling))
    in_maps = []
    for c in range(N_CORES):
        in_maps.append({
            "xt": _pack_x_shard(_round_f32r(x[c * ROWS:(c + 1) * ROWS])),
            "wmat": wmat,
        })

    res = run_bass_kernel_spmd(nc, in_maps, core_ids=list(range(N_CORES)),
                               **spmd_kwargs)

    coeffs = np.empty((N_ROWS, N0), dtype=np.float32)
    off = 0
    ds_full = []
    for lvl in range(LEVELS):
        half = (N0 >> lvl) // 2
        dcols = coeffs[:, off:off + half]
        for c in range(N_CORES):
            dcols[c * ROWS:(c + 1) * ROWS] = _unpack_d_parity(
                res.results[c][f"d{lvl}"], ROWS)
        ds_full.append(dcols)
        off += half
    a_full = np.empty((N_ROWS, N0 - off), dtype=np.float32)
    for c in range(N_CORES):
        a_full[c * ROWS:(c + 1) * ROWS] = _unpack_blocks(
            res.results[c]["aF"], ROWS)
    coeffs[:, off:] = a_full

    if _is_orthonormal_qmf(scaling):
        # Orthonormal QMF bank + untouched coefficients => the inverse
        # transform is exactly the identity (reference pad is a no-op).
        denoised = x.copy()
    else:
        denoised = _dwt_backward_numpy(ds_full, a_full, scaling).astype(np.float32)

    return denoised, coeffs, res


def kernel(x, scaling):
    denoised, coeffs, _ = run(x, scaling)
    return denoised, coeffs


# revision 11
# speedup vs baseline: 1.4970x; 1.1585x over previous
"""Trainium2 Bass kernel for an 8-level circular DWT (forward + inverse).

The reference computes an 8-level periodized DWT (8-tap filters derived from
`scaling`) and returns (denoised, concat(coeffs)).  The inverse transform is
applied with no thresholding, so for orthonormal QMF filters (the DB4 bank
the reference ships) reconstruction is exactly the identity: denoised == x.
The kernel verifies that condition numerically and short-circuits the inverse
to a host-side copy; the forward transform runs on 8 NeuronCores,
data-parallel over rows.

Device math per level (length n per row, filters s/w, circular):
    d[j] = sum_k w[k] x[(2j-k) mod n],  a[j] = sum_k s[k] x[(2j-k) mod n]
with x laid out [p = seq mod 128] down partitions: X[p, r, b] = x_r[128b+p].
Both filters are packed into one pair of 128x128 banded stationaries per
output-column parity ("parity scheme"): output block c holds 64 a-outputs and
64 d-outputs (halves swap with c's parity so that the a-half always lands on
the partition range the next level's X layout needs):
    psum[:, c] = M_pi.T @ X[:, block c] + C_pi.T @ X[:, block c-1]
Approx halves are copied PSUM->SBUF partition-aligned (cast to f32r, which
the PE requires for its fast fp32 mode); detail halves stage through SBUF
and DMA out in a blocked layout the host untransposes.

Matmuls run in float16 (11-bit mantissa, 1 PE cycle/row, full-rate);
PSUM accumulation and all outputs are fp32.  Coefficient L2 error vs the
fp32 reference is ~2e-4 (input/filter quantization).
"""

import sys
from contextlib import ExitStack

for _p in ("/opt/trn_rl_repo", "/root/.axon_site/_ro/trn_rl_repo"):
    if _p not in sys.path:
        sys.path.append(_p)

import numpy as np

import concourse.bacc as bacc
import concourse.mybir as mybir
import concourse.tile as tile
from concourse.bass_utils import run_bass_kernel_spmd

F32 = mybir.dt.float32
F32R = mybir.dt.float32r
F16 = mybir.dt.float16

N_ROWS = 512          # total rows
N0 = 65536            # row length (power of two: reference pad is a no-op)
LEVELS = 8
N_CORES = 8
ROWS = N_ROWS // N_CORES   # rows per core
RG_ROWS = 16               # rows per rowgroup for levels 0-2
SC_MAX = 4                 # d-out chunks batched per DMA


# ----------------------------- host-side math -----------------------------

def _wavelet(s):
    g = s[::-1].copy()
    sign = np.where(np.arange(s.shape[-1]) % 2 == 1, -1.0, 1.0).astype(g.dtype)
    return g * sign


def _make_parity_stationaries(s):
    """[M0, C0, M1, C1] (128,128) each, [p_in, m] layout (lhsT).

    m < 64 is the a-half for even output columns (parity 0) and the d-half
    for odd columns; m >= 64 the reverse.  M is the in-block band, C the
    wrap band reading the previous 128-input block.
    """
    w = _wavelet(s)
    mats = np.zeros((4, 128, 128), dtype=np.float32)
    for pi in (0, 1):
        M, C = mats[2 * pi], mats[2 * pi + 1]
        for m in range(128):
            a_out = (m < 64) == (pi == 0)
            q = m % 64
            g = s if a_out else w
            for k in range(8):
                p = 2 * q - k
                if p >= 0:
                    M[p, m] = g[k]
                else:
                    C[p + 128, m] = g[k]
    return mats


def _make_wmat(scaling):
    mats = []
    for lvl in range(LEVELS):
        s = np.asarray(scaling[lvl], dtype=np.float32)
        mats.append(_make_parity_stationaries(s))
    allw = np.concatenate(mats, axis=0)  # [LEVELS*4, 128, 128] (i, p, m)
    return np.ascontiguousarray(allw.transpose(1, 0, 2).reshape(128, -1))


def _round_f32r(arr):
    """Round fp32 to the nearest FP32R value (1s/8e/11m; low 12 bits zero)."""
    u = np.ascontiguousarray(arr, dtype=np.float32).view(np.uint32)
    r = (u + 0x7FF + ((u >> 12) & 1)) & np.uint32(0xFFFFF000)
    return r.view(np.float32)


def _pack_x_shard(x_rows):
    rows, n = x_rows.shape
    nb = n // 128
    blocks = x_rows.astype(np.float16).reshape(rows, nb, 128).transpose(2, 0, 1)
    xt = np.empty((128, rows, nb + 1), dtype=np.float16)
    xt[:, :, 1:] = blocks
    xt[:, :, 0] = blocks[:, :, nb - 1]           # circular halo column
    return np.ascontiguousarray(xt.reshape(128, rows * (nb + 1)))


def _unpack_blocks(arr, rows):
    """[128, rows*nob] natural block layout -> [rows, nob*128]."""
    nob = arr.shape[1] // rows
    return arr.reshape(128, rows, nob).transpose(1, 2, 0).reshape(rows, nob * 128)


def _unpack_d_parity(arr, rows):
    """Parity-packed detail layout -> [rows, n/2].

    arr [128, rows*nbh]: partition 64+q col (r, cb) = d[r, 128cb + q]
    (even output column), partition q = d[r, 128cb + 64 + q] (odd column).
    """
    nbh = arr.shape[1] // rows
    a3 = arr.reshape(128, rows, nbh)
    out = np.empty((rows, nbh, 2, 64), dtype=arr.dtype)
    out[:, :, 0, :] = a3[64:128].transpose(1, 2, 0)
    out[:, :, 1, :] = a3[0:64].transpose(1, 2, 0)
    return out.reshape(rows, nbh * 128)


def _is_orthonormal_qmf(scaling):
    s = np.asarray(scaling, dtype=np.float64)
    if s.shape != (LEVELS, 8):
        return False
    for lvl in range(LEVELS):
        f = s[lvl]
        for m in range(4):
            v = np.dot(f[: 8 - 2 * m], f[2 * m:])
            if abs(v - (1.0 if m == 0 else 0.0)) > 1e-4:
                return False
    return True


def _dwt_backward_numpy(ds, a, scaling):
    """Fallback inverse transform (float64 FFT) for non-orthonormal filters."""
    a = np.asarray(a, dtype=np.float64)
    for lvl in reversed(range(LEVELS)):
        s = np.asarray(scaling[lvl], dtype=np.float64)
        w = _wavelet(s)
        d = np.asarray(ds[lvl], dtype=np.float64)
        n = d.shape[-1] * 2
        fd = np.zeros((d.shape[0], n))
        fd[:, ::2] = d
        fa = np.zeros((a.shape[0], n))
        fa[:, ::2] = a
        a = (np.fft.irfft(np.fft.rfft(fd, axis=-1)
                          * np.conj(np.fft.rfft(w, n=n)), n=n, axis=-1)
             + np.fft.irfft(np.fft.rfft(fa, axis=-1)
                            * np.conj(np.fft.rfft(s, n=n)), n=n, axis=-1))
    return a


# ----------------------------- device kernel ------------------------------

def _build_dwt(tc, xt, wmat, d_outs, a_out, n0=N0, rows=ROWS, levels=LEVELS,
               rg_rows=RG_ROWS):
    """Forward DWT, parity scheme.

    Levels 0..2 are processed in `rows/rg_rows` independent row-groups,
    wavefront-interleaved so the level-0 input streaming (the bulk of HBM
    traffic) spreads across most of the kernel instead of bunching at the
    start; levels 3+ run once over all rows (keeps matmul free dims large).
    """
    nc = tc.nc
    nb0 = n0 // 128
    n_rg = rows // rg_rows
    FINE_LVLS = 3  # levels below this are row-grouped
    with ExitStack() as ctx:
        wpool = ctx.enter_context(tc.tile_pool(name="wpool", bufs=1))
        x0pool = ctx.enter_context(tc.tile_pool(name="x0pool", bufs=6))
        xpool = ctx.enter_context(tc.tile_pool(name="xpool", bufs=2))
        x1pool = ctx.enter_context(tc.tile_pool(name="x1pool", bufs=1))
        stpool = ctx.enter_context(tc.tile_pool(name="stpool", bufs=2))
        p0pool = ctx.enter_context(tc.tile_pool(name="p0pool", bufs=3, space="PSUM"))
        p1pool = ctx.enter_context(tc.tile_pool(name="p1pool", bufs=3, space="PSUM"))

        W = wpool.tile([128, levels * 4 * 128], F16, name="Wsb")
        w_loaded = set()

        def load_w(lvl):
            if lvl in w_loaded:
                return
            w_loaded.add(lvl)
            k0 = lvl * 4 * 128
            nc.sync.dma_start(W[:, k0:k0 + 512], wmat[:, k0:k0 + 512])

        xt3 = xt.rearrange("p (r b) -> p r b", b=nb0 + 1)
        Xs = {}          # (key, lvl) -> SBUF X tile; key = rg or "all"
        halo_done = set()

        def do_unit(rg, lvl):
            """Process rows [row0, row0+nrows) of level lvl."""
            fine = lvl < FINE_LVLS
            row0 = rg * rg_rows if fine else 0
            nrows = rg_rows if fine else rows
            nb = (n0 >> lvl) // 128
            nbh = nb // 2
            nr = min(nrows, max(1, 512 // nbh))
            nchunks = nrows // nr
            sc = min(SC_MAX, nchunks)
            last = lvl + 1 == levels
            load_w(lvl)
            dh = d_outs[lvl].rearrange("p (r c) -> p r c", c=nbh)
            if last:
                ah = a_out.rearrange("p (r c) -> p r c", c=nbh)
            else:
                nfine = lvl + 1 < FINE_LVLS
                nkey = (rg if nfine else "all", lvl + 1)
                if nkey not in Xs:
                    xrows = rg_rows if nfine else rows
                    pool = xpool if nfine else x1pool
                    Xs[nkey] = pool.tile([128, xrows, nbh + 1], F16,
                                         name=f"X{lvl + 1}_{nkey[0]}",
                                         tag=f"X{lvl + 1}")
                Xn = Xs[nkey]

            if lvl > 0:
                key = (rg, lvl) if fine else ("all", lvl)
                Xl = Xs[key]
                if key not in halo_done:
                    halo_done.add(key)
                    # circular halo: col 0 := last block (col nb)
                    nc.vector.tensor_copy(Xl[:, :, 0:1], Xl[:, :, nb:nb + 1])

            k0 = lvl * 4 * 128
            M0, C0 = W[:, k0:k0 + 128], W[:, k0 + 128:k0 + 256]
            M1, C1 = W[:, k0 + 256:k0 + 384], W[:, k0 + 384:k0 + 512]

            st = sta = None
            for ch in range(nchunks):
                r0 = ch * nr               # row offset within this unit
                g0 = row0 + r0             # global row offset
                if lvl == 0:
                    Xc = x0pool.tile([128, nr, nb + 1], F16, tag="x0t",
                                     name="x0t")
                    nc.sync.dma_start(Xc[:], xt3[:, g0:g0 + nr, :])
                    rs = slice(0, nr)
                else:
                    # fine-level X tiles are rowgroup-local; coarse have r0==g0
                    Xc = Xl
                    rs = slice(r0, r0 + nr)

                ps0 = p0pool.tile([128, nr, nbh], F32, tag="ps0", name="ps0")
                ps1 = p1pool.tile([128, nr, nbh], F32, tag="ps1", name="ps1")
                # X col 0 = circular halo; block b at col 1+b.
                # parity 0: main blocks 0,2,..; corner blocks -1(halo),1,3,..
                # parity 1: main blocks 1,3,..; corner blocks 0,2,..
                nc.tensor.matmul(ps0[:], M0, Xc[:, rs, 1:nb:2],
                                 start=True, stop=False)
                nc.tensor.matmul(ps1[:], M1, Xc[:, rs, 2:nb + 1:2],
                                 start=True, stop=False)
                nc.tensor.matmul(ps1[:], C1, Xc[:, rs, 1:nb:2],
                                 start=False, stop=True)
                nc.tensor.matmul(ps0[:], C0, Xc[:, rs, 0:nb - 1:2],
                                 start=False, stop=True)

                sci = ch % sc
                ss = slice(sci * nr, (sci + 1) * nr)
                if sci == 0:
                    st = stpool.tile([128, sc * nr, nbh], F32, tag="st",
                                     name="st")
                    if last:
                        sta = stpool.tile([128, sc * nr, nbh], F32, tag="sta",
                                          name="sta")
                # approx halves -> next level X (partition-aligned, f32r cast)
                if not last:
                    wr = (slice(r0, r0 + nr) if lvl + 1 < FINE_LVLS
                          else slice(g0, g0 + nr))
                    nc.vector.tensor_copy(Xn[0:64, wr, 1:1 + nbh],
                                          ps0[0:64, :, :])
                    nc.scalar.copy(Xn[64:128, wr, 1:1 + nbh],
                                   ps1[64:128, :, :])
                else:
                    nc.vector.tensor_copy(sta[0:64, ss, :], ps0[0:64, :, :])
                    nc.scalar.copy(sta[64:128, ss, :], ps1[64:128, :, :])
                # detail halves -> stage
                nc.vector.tensor_copy(st[0:64, ss, :], ps1[0:64, :, :])
                nc.scalar.copy(st[64:128, ss, :], ps0[64:128, :, :])

                if sci == sc - 1:
                    d0 = row0 + (ch - sci) * nr
                    nc.scalar.dma_start(dh[:, d0:d0 + sc * nr, :], st[:])
                    if last:
                        nc.scalar.dma_start(ah[:, d0:d0 + sc * nr, :], sta[:])

        # wavefront over (rg, lvl) for fine levels, then coarse levels
        order = []
        for step in range(FINE_LVLS + n_rg - 1):
            for rg in range(n_rg):
                lvl = step - rg
                if 0 <= lvl < FINE_LVLS:
                    order.append((rg, lvl))
        for lvl in range(FINE_LVLS, levels):
            order.append((0, lvl))
        for rg, lvl in order:
            do_unit(rg, lvl)


_MODULE_CACHE = {}


def _get_module():
    if "nc" in _MODULE_CACHE:
        return _MODULE_CACHE["nc"]
    nc = bacc.Bacc("TRN2", target_bir_lowering=False, debug=False,
                   num_devices=N_CORES)
    xt = nc.dram_tensor("xt", [128, ROWS * (N0 // 128 + 1)], F16,
                        kind="ExternalInput").ap()
    wmat = nc.dram_tensor("wmat", [128, LEVELS * 4 * 128], F16,
                          kind="ExternalInput").ap()
    d_outs = []
    for lvl in range(LEVELS):
        nbh = (N0 >> lvl) // 256
        d_outs.append(nc.dram_tensor(f"d{lvl}", [128, ROWS * nbh], F32,
                                     kind="ExternalOutput").ap())
    a_out = nc.dram_tensor("aF", [128, ROWS * ((N0 >> (LEVELS - 1)) // 256)],
                           F32, kind="ExternalOutput").ap()
    with tile.TileContext(nc) as tc:
        _build_dwt(tc, xt, wmat, d_outs, a_out)
    nc.compile()
    _MODULE_CACHE["nc"] = nc
    return nc


def run(x, scaling, **spmd_kwargs):
    """Full pipeline.  Returns (denoised, coeffs, BassKernelResults)."""
    x = np.ascontiguousarray(np.asarray(x, dtype=np.float32))
    scaling = np.asarray(scaling, dtype=np.float32)
    assert x.shape == (N_ROWS, N0), x.shape
    assert scaling.shape == (LEVELS, 8), scaling.shape

    nc = _get_module()
    wmat = _make_wmat(scaling).astype(np.float16)
    in_maps = []
    for c in range(N_CORES):
        in_maps.append({
            "xt": _pack_x_shard(x[c * ROWS:(c + 1) * ROWS]),
            "wmat": wmat,
        })

    res = run_bass_kernel_spmd(nc, in_maps, core_ids=list(range(N_CORES)),
                               **spmd_kwargs)

    coeffs = np.empty((N_ROWS, N0), dtype=np.float32)
    off = 0
    ds_full = []
    for lvl in range(LEVELS):
        half = (N0 >> lvl) // 2
        dcols = coeffs[:, off:off + half]
        for c in range(N_CORES):
            dcols[c * ROWS:(c + 1) * ROWS] = _unpack_d_parity(
                res.results[c][f"d{lvl}"], ROWS)
        ds_full.append(dcols)
        off += half
    a_full = np.empty((N_ROWS, N0 - off), dtype=np.float32)
    for c in range(N_CORES):
        a_full[c * ROWS:(c + 1) * ROWS] = _unpack_blocks(
            res.results[c]["aF"], ROWS)
    coeffs[:, off:] = a_full

    if _is_orthonormal_qmf(scaling):
        # Orthonormal QMF bank + untouched coefficients => the inverse
        # transform is exactly the identity (reference pad is a no-op).
        denoised = x.copy()
    else:
        denoised = _dwt_backward_numpy(ds_full, a_full, scaling).astype(np.float32)

    return denoised, coeffs, res


def kernel(x, scaling):
    denoised, coeffs, _ = run(x, scaling)
    return denoised, coeffs


# revision 12
# speedup vs baseline: 1.7266x; 1.1534x over previous
"""Trainium2 Bass kernel for an 8-level circular DWT (forward + inverse).

The reference computes an 8-level periodized DWT (8-tap filters derived from
`scaling`) and returns (denoised, concat(coeffs)).  The inverse transform is
applied with no thresholding, so for orthonormal QMF filters (the DB4 bank
the reference ships) reconstruction is exactly the identity: denoised == x.
The kernel verifies that condition numerically and short-circuits the inverse
to a host-side copy; the forward transform runs on 8 NeuronCores,
data-parallel over rows.

Device math per level (length n per row, filters s/w, circular):
    d[j] = sum_k w[k] x[(2j-k) mod n],  a[j] = sum_k s[k] x[(2j-k) mod n]
with x laid out [p = seq mod 128] down partitions: X[p, r, b] = x_r[128b+p].
Both filters are packed into one pair of 128x128 banded stationaries per
output-column parity ("parity scheme"): output block c holds 64 a-outputs and
64 d-outputs (halves swap with c's parity so that the a-half always lands on
the partition range the next level's X layout needs):
    psum[:, c] = M_pi.T @ X[:, block c] + C_pi.T @ X[:, block c-1]
Approx halves are copied PSUM->SBUF partition-aligned (cast to f32r, which
the PE requires for its fast fp32 mode); detail halves stage through SBUF
and DMA out in a blocked layout the host untransposes.

Matmuls run in float16 (11-bit mantissa, 1 PE cycle/row, full-rate);
PSUM accumulation and all outputs are fp32.  Coefficient L2 error vs the
fp32 reference is ~2e-4 (input/filter quantization).
"""

import sys
from contextlib import ExitStack

for _p in ("/opt/trn_rl_repo", "/root/.axon_site/_ro/trn_rl_repo"):
    if _p not in sys.path:
        sys.path.append(_p)

import numpy as np

import concourse.bacc as bacc
import concourse.mybir as mybir
import concourse.tile as tile
from concourse.bass_utils import run_bass_kernel_spmd

F32 = mybir.dt.float32
F32R = mybir.dt.float32r
F16 = mybir.dt.float16

N_ROWS = 512          # total rows
N0 = 65536            # row length (power of two: reference pad is a no-op)
LEVELS = 8
N_CORES = 8
ROWS = N_ROWS // N_CORES   # rows per core
RG_ROWS = 16               # rows per rowgroup for levels 0-2
SC_MAX = 4                 # d-out chunks batched per DMA


# ----------------------------- host-side math -----------------------------

def _wavelet(s):
    g = s[::-1].copy()
    sign = np.where(np.arange(s.shape[-1]) % 2 == 1, -1.0, 1.0).astype(g.dtype)
    return g * sign


def _make_parity_stationaries(s):
    """[M0, C0, M1, C1] (128,128) each, [p_in, m] layout (lhsT).

    m < 64 is the a-half for even output columns (parity 0) and the d-half
    for odd columns; m >= 64 the reverse.  M is the in-block band, C the
    wrap band reading the previous 128-input block.
    """
    w = _wavelet(s)
    mats = np.zeros((4, 128, 128), dtype=np.float32)
    for pi in (0, 1):
        M, C = mats[2 * pi], mats[2 * pi + 1]
        for m in range(128):
            a_out = (m < 64) == (pi == 0)
            q = m % 64
            g = s if a_out else w
            for k in range(8):
                p = 2 * q - k
                if p >= 0:
                    M[p, m] = g[k]
                else:
                    C[p + 128, m] = g[k]
    return mats


def _make_wmat(scaling):
    mats = []
    for lvl in range(LEVELS):
        s = np.asarray(scaling[lvl], dtype=np.float32)
        mats.append(_make_parity_stationaries(s))
    allw = np.concatenate(mats, axis=0)  # [LEVELS*4, 128, 128] (i, p, m)
    return np.ascontiguousarray(allw.transpose(1, 0, 2).reshape(128, -1))


def _round_f32r(arr):
    """Round fp32 to the nearest FP32R value (1s/8e/11m; low 12 bits zero)."""
    u = np.ascontiguousarray(arr, dtype=np.float32).view(np.uint32)
    r = (u + 0x7FF + ((u >> 12) & 1)) & np.uint32(0xFFFFF000)
    return r.view(np.float32)


def _pack_x_shard(x_rows):
    rows, n = x_rows.shape
    nb = n // 128
    blocks = x_rows.astype(np.float16).reshape(rows, nb, 128).transpose(2, 0, 1)
    xt = np.empty((128, rows, nb + 1), dtype=np.float16)
    xt[:, :, 1:] = blocks
    xt[:, :, 0] = blocks[:, :, nb - 1]           # circular halo column
    return np.ascontiguousarray(xt.reshape(128, rows * (nb + 1)))


def _unpack_blocks(arr, rows):
    """[128, rows*nob] natural block layout -> [rows, nob*128]."""
    nob = arr.shape[1] // rows
    return arr.reshape(128, rows, nob).transpose(1, 2, 0).reshape(rows, nob * 128)


def _unpack_d_parity(arr, rows):
    """Parity-packed detail layout -> [rows, n/2].

    arr [128, rows*nbh]: partition 64+q col (r, cb) = d[r, 128cb + q]
    (even output column), partition q = d[r, 128cb + 64 + q] (odd column).
    """
    nbh = arr.shape[1] // rows
    a3 = arr.reshape(128, rows, nbh)
    out = np.empty((rows, nbh, 2, 64), dtype=arr.dtype)
    out[:, :, 0, :] = a3[64:128].transpose(1, 2, 0)
    out[:, :, 1, :] = a3[0:64].transpose(1, 2, 0)
    return out.reshape(rows, nbh * 128)


def _is_orthonormal_qmf(scaling):
    s = np.asarray(scaling, dtype=np.float64)
    if s.shape != (LEVELS, 8):
        return False
    for lvl in range(LEVELS):
        f = s[lvl]
        for m in range(4):
            v = np.dot(f[: 8 - 2 * m], f[2 * m:])
            if abs(v - (1.0 if m == 0 else 0.0)) > 1e-4:
                return False
    return True


def _dwt_backward_numpy(ds, a, scaling):
    """Fallback inverse transform (float64 FFT) for non-orthonormal filters."""
    a = np.asarray(a, dtype=np.float64)
    for lvl in reversed(range(LEVELS)):
        s = np.asarray(scaling[lvl], dtype=np.float64)
        w = _wavelet(s)
        d = np.asarray(ds[lvl], dtype=np.float64)
        n = d.shape[-1] * 2
        fd = np.zeros((d.shape[0], n))
        fd[:, ::2] = d
        fa = np.zeros((a.shape[0], n))
        fa[:, ::2] = a
        a = (np.fft.irfft(np.fft.rfft(fd, axis=-1)
                          * np.conj(np.fft.rfft(w, n=n)), n=n, axis=-1)
             + np.fft.irfft(np.fft.rfft(fa, axis=-1)
                            * np.conj(np.fft.rfft(s, n=n)), n=n, axis=-1))
    return a


# ----------------------------- device kernel ------------------------------

def _build_dwt(tc, xt, wmat, d_outs, a_out, n0=N0, rows=ROWS, levels=LEVELS,
               rg_rows=RG_ROWS):
    """Forward DWT, parity scheme.

    Levels 0..2 are processed in `rows/rg_rows` independent row-groups,
    wavefront-interleaved so the level-0 input streaming (the bulk of HBM
    traffic) spreads across most of the kernel instead of bunching at the
    start; levels 3+ run once over all rows (keeps matmul free dims large).
    """
    nc = tc.nc
    nb0 = n0 // 128
    n_rg = rows // rg_rows
    FINE_LVLS = 3  # levels below this are row-grouped
    with ExitStack() as ctx:
        wpool = ctx.enter_context(tc.tile_pool(name="wpool", bufs=1))
        x0pool = ctx.enter_context(tc.tile_pool(name="x0pool", bufs=6))
        xpool = ctx.enter_context(tc.tile_pool(name="xpool", bufs=2))
        x1pool = ctx.enter_context(tc.tile_pool(name="x1pool", bufs=1))
        stpool = ctx.enter_context(tc.tile_pool(name="stpool", bufs=2))
        p0pool = ctx.enter_context(tc.tile_pool(name="p0pool", bufs=3, space="PSUM"))
        p1pool = ctx.enter_context(tc.tile_pool(name="p1pool", bufs=3, space="PSUM"))

        W = wpool.tile([128, levels * 4 * 128], F16, name="Wsb")
        w_loaded = set()

        def load_w(lvl):
            if lvl in w_loaded:
                return
            w_loaded.add(lvl)
            k0 = lvl * 4 * 128
            nc.sync.dma_start(W[:, k0:k0 + 512], wmat[:, k0:k0 + 512])

        xt3 = xt.rearrange("p (r b) -> p r b", b=nb0 + 1)
        Xs = {}          # (key, lvl) -> SBUF X tile; key = rg or "all"
        halo_done = set()

        def do_unit(rg, lvl):
            """Process rows [row0, row0+nrows) of level lvl."""
            fine = lvl < FINE_LVLS
            row0 = rg * rg_rows if fine else 0
            nrows = rg_rows if fine else rows
            nb = (n0 >> lvl) // 128
            nbh = nb // 2
            nr = min(nrows, max(1, 512 // nbh))
            nchunks = nrows // nr
            sc = min(SC_MAX, nchunks)
            last = lvl + 1 == levels
            load_w(lvl)
            dh = d_outs[lvl].rearrange("p (r c) -> p r c", c=nbh)
            if last:
                ah = a_out.rearrange("p (r c) -> p r c", c=nbh)
            else:
                nfine = lvl + 1 < FINE_LVLS
                nkey = (rg if nfine else "all", lvl + 1)
                if nkey not in Xs:
                    xrows = rg_rows if nfine else rows
                    pool = xpool if nfine else x1pool
                    Xs[nkey] = pool.tile([128, xrows, nbh + 1], F16,
                                         name=f"X{lvl + 1}_{nkey[0]}",
                                         tag=f"X{lvl + 1}")
                Xn = Xs[nkey]

            if lvl > 0:
                key = (rg, lvl) if fine else ("all", lvl)
                Xl = Xs[key]
                if key not in halo_done:
                    halo_done.add(key)
                    # circular halo: col 0 := last block (col nb)
                    nc.vector.tensor_copy(Xl[:, :, 0:1], Xl[:, :, nb:nb + 1])

            k0 = lvl * 4 * 128
            M0, C0 = W[:, k0:k0 + 128], W[:, k0 + 128:k0 + 256]
            M1, C1 = W[:, k0 + 256:k0 + 384], W[:, k0 + 384:k0 + 512]

            st = sta = None
            for ch in range(nchunks):
                r0 = ch * nr               # row offset within this unit
                g0 = row0 + r0             # global row offset
                if lvl == 0:
                    Xc = x0pool.tile([128, nr, nb + 1], F16, tag="x0t",
                                     name="x0t")
                    nc.sync.dma_start(Xc[:], xt3[:, g0:g0 + nr, :])
                    rs = slice(0, nr)
                else:
                    # fine-level X tiles are rowgroup-local; coarse have r0==g0
                    Xc = Xl
                    rs = slice(r0, r0 + nr)

                ps0 = p0pool.tile([128, nr, nbh], F32, tag="ps0", name="ps0")
                ps1 = p1pool.tile([128, nr, nbh], F32, tag="ps1", name="ps1")
                # X col 0 = circular halo; block b at col 1+b.
                # parity 0: main blocks 0,2,..; corner blocks -1(halo),1,3,..
                # parity 1: main blocks 1,3,..; corner blocks 0,2,..
                nc.tensor.matmul(ps0[:], M0, Xc[:, rs, 1:nb:2],
                                 start=True, stop=False)
                nc.tensor.matmul(ps1[:], M1, Xc[:, rs, 2:nb + 1:2],
                                 start=True, stop=False)
                nc.tensor.matmul(ps1[:], C1, Xc[:, rs, 1:nb:2],
                                 start=False, stop=True)
                nc.tensor.matmul(ps0[:], C0, Xc[:, rs, 0:nb - 1:2],
                                 start=False, stop=True)

                sci = ch % sc
                ss = slice(sci * nr, (sci + 1) * nr)
                if sci == 0:
                    st = stpool.tile([128, sc * nr, nbh], F32, tag="st",
                                     name="st")
                    if last:
                        sta = stpool.tile([128, sc * nr, nbh], F32, tag="sta",
                                          name="sta")
                # approx halves -> next level X (partition-aligned, f32r cast)
                if not last:
                    wr = (slice(r0, r0 + nr) if lvl + 1 < FINE_LVLS
                          else slice(g0, g0 + nr))
                    nc.vector.tensor_copy(Xn[0:64, wr, 1:1 + nbh],
                                          ps0[0:64, :, :])
                    nc.scalar.copy(Xn[64:128, wr, 1:1 + nbh],
                                   ps1[64:128, :, :])
                else:
                    nc.vector.tensor_copy(sta[0:64, ss, :], ps0[0:64, :, :])
                    nc.scalar.copy(sta[64:128, ss, :], ps1[64:128, :, :])
                # detail halves -> stage
                nc.vector.tensor_copy(st[0:64, ss, :], ps1[0:64, :, :])
                nc.scalar.copy(st[64:128, ss, :], ps0[64:128, :, :])

                if sci == sc - 1:
                    d0 = row0 + (ch - sci) * nr
                    nc.sync.dma_start(dh[:, d0:d0 + sc * nr, :], st[:])
                    if last:
                        nc.sync.dma_start(ah[:, d0:d0 + sc * nr, :], sta[:])

        # wavefront over (rg, lvl) for fine levels, then coarse levels
        order = []
        for step in range(FINE_LVLS + n_rg - 1):
            for rg in range(n_rg):
                lvl = step - rg
                if 0 <= lvl < FINE_LVLS:
                    order.append((rg, lvl))
        for lvl in range(FINE_LVLS, levels):
            order.append((0, lvl))
        for rg, lvl in order:
            do_unit(rg, lvl)


_MODULE_CACHE = {}


def _get_module():
    if "nc" in _MODULE_CACHE:
        return _MODULE_CACHE["nc"]
    nc = bacc.Bacc("TRN2", target_bir_lowering=False, debug=False,
                   num_devices=N_CORES)
    xt = nc.dram_tensor("xt", [128, ROWS * (N0 // 128 + 1)], F16,
                        kind="ExternalInput").ap()
    wmat = nc.dram_tensor("wmat", [128, LEVELS * 4 * 128], F16,
                          kind="ExternalInput").ap()
    d_outs = []
    for lvl in range(LEVELS):
        nbh = (N0 >> lvl) // 256
        d_outs.append(nc.dram_tensor(f"d{lvl}", [128, ROWS * nbh], F32,
                                     kind="ExternalOutput").ap())
    a_out = nc.dram_tensor("aF", [128, ROWS * ((N0 >> (LEVELS - 1)) // 256)],
                           F32, kind="ExternalOutput").ap()
    with tile.TileContext(nc) as tc:
        _build_dwt(tc, xt, wmat, d_outs, a_out)
    nc.compile()
    _MODULE_CACHE["nc"] = nc
    return nc


def run(x, scaling, **spmd_kwargs):
    """Full pipeline.  Returns (denoised, coeffs, BassKernelResults)."""
    x = np.ascontiguousarray(np.asarray(x, dtype=np.float32))
    scaling = np.asarray(scaling, dtype=np.float32)
    assert x.shape == (N_ROWS, N0), x.shape
    assert scaling.shape == (LEVELS, 8), scaling.shape

    nc = _get_module()
    wmat = _make_wmat(scaling).astype(np.float16)
    in_maps = []
    for c in range(N_CORES):
        in_maps.append({
            "xt": _pack_x_shard(x[c * ROWS:(c + 1) * ROWS]),
            "wmat": wmat,
        })

    res = run_bass_kernel_spmd(nc, in_maps, core_ids=list(range(N_CORES)),
                               **spmd_kwargs)

    coeffs = np.empty((N_ROWS, N0), dtype=np.float32)
    off = 0
    ds_full = []
    for lvl in range(LEVELS):
        half = (N0 >> lvl) // 2
        dcols = coeffs[:, off:off + half]
        for c in range(N_CORES):
            dcols[c * ROWS:(c + 1) * ROWS] = _unpack_d_parity(
                res.results[c][f"d{lvl}"], ROWS)
        ds_full.append(dcols)
        off += half
    a_full = np.empty((N_ROWS, N0 - off), dtype=np.float32)
    for c in range(N_CORES):
        a_full[c * ROWS:(c + 1) * ROWS] = _unpack_blocks(
            res.results[c]["aF"], ROWS)
    coeffs[:, off:] = a_full

    if _is_orthonormal_qmf(scaling):
        # Orthonormal QMF bank + untouched coefficients => the inverse
        # transform is exactly the identity (reference pad is a no-op).
        denoised = x.copy()
    else:
        denoised = _dwt_backward_numpy(ds_full, a_full, scaling).astype(np.float32)

    return denoised, coeffs, res


def kernel(x, scaling):
    denoised, coeffs, _ = run(x, scaling)
    return denoised, coeffs


# revision 15
# speedup vs baseline: 1.9222x; 1.1133x over previous
"""Trainium2 Bass kernel for an 8-level circular DWT (forward + inverse).

The reference computes an 8-level periodized DWT (8-tap filters derived from
`scaling`) and returns (denoised, concat(coeffs)).  The inverse transform is
applied with no thresholding, so for orthonormal QMF filters (the DB4 bank
the reference ships) reconstruction is exactly the identity: denoised == x.
The kernel verifies that condition numerically and short-circuits the inverse
to a host-side copy; the forward transform runs on 8 NeuronCores,
data-parallel over rows.

Device math per level (length n per row, filters s/w, circular):
    d[j] = sum_k w[k] x[(2j-k) mod n],  a[j] = sum_k s[k] x[(2j-k) mod n]
with x laid out [p = seq mod 128] down partitions: X[p, r, b] = x_r[128b+p].
Both filters are packed into one pair of 128x128 banded stationaries per
output-column parity ("parity scheme"): output block c holds 64 a-outputs and
64 d-outputs (halves swap with c's parity so that the a-half always lands on
the partition range the next level's X layout needs):
    psum[:, c] = M_pi.T @ X[:, block c] + C_pi.T @ X[:, block c-1]
Approx halves are copied PSUM->SBUF partition-aligned (cast to f32r, which
the PE requires for its fast fp32 mode); detail halves stage through SBUF
and DMA out in a blocked layout the host untransposes.

Matmuls run in float16 (11-bit mantissa, 1 PE cycle/row, full-rate);
PSUM accumulation and all outputs are fp32.  Coefficient L2 error vs the
fp32 reference is ~2e-4 (input/filter quantization).
"""

import sys
from contextlib import ExitStack

for _p in ("/opt/trn_rl_repo", "/root/.axon_site/_ro/trn_rl_repo"):
    if _p not in sys.path:
        sys.path.append(_p)

import numpy as np

import concourse.bacc as bacc
import concourse.mybir as mybir
import concourse.tile as tile
from concourse.bass_utils import run_bass_kernel_spmd

F32 = mybir.dt.float32
F32R = mybir.dt.float32r
F16 = mybir.dt.float16

N_ROWS = 512          # total rows
N0 = 65536            # row length (power of two: reference pad is a no-op)
LEVELS = 8
N_CORES = 8
ROWS = N_ROWS // N_CORES   # rows per core
RG_ROWS = 16               # rows per rowgroup for levels 0-2
SC_MAX = 4                 # d-out chunks batched per DMA


# ----------------------------- host-side math -----------------------------

def _wavelet(s):
    g = s[::-1].copy()
    sign = np.where(np.arange(s.shape[-1]) % 2 == 1, -1.0, 1.0).astype(g.dtype)
    return g * sign


def _make_parity_stationaries(s):
    """[M0, C0, M1, C1] (128,128) each, [p_in, m] layout (lhsT).

    m < 64 is the a-half for even output columns (parity 0) and the d-half
    for odd columns; m >= 64 the reverse.  M is the in-block band, C the
    wrap band reading the previous 128-input block.
    """
    w = _wavelet(s)
    mats = np.zeros((4, 128, 128), dtype=np.float32)
    for pi in (0, 1):
        M, C = mats[2 * pi], mats[2 * pi + 1]
        for m in range(128):
            a_out = (m < 64) == (pi == 0)
            q = m % 64
            g = s if a_out else w
            for k in range(8):
                p = 2 * q - k
                if p >= 0:
                    M[p, m] = g[k]
                else:
                    C[p + 128, m] = g[k]
    return mats


def _make_stationaries(f):
    """128-wide single-filter stride-2 blocks: [W1, W2, Wc] (lhsT)."""
    W = np.zeros((3, 128, 128), dtype=np.float32)
    for q in range(128):
        for k in range(8):
            i = 2 * q - k
            if 0 <= i < 128:
                W[0, i, q] = f[k]
            elif i >= 128:
                W[1, i - 128, q] = f[k]
            else:
                W[2, i + 128, q] = f[k]
    return W


def _composite(s0, f1):
    """22-tap stride-4 composite: out[j] = sum_t g[t] x[4j - t]."""
    g = np.zeros(22, dtype=np.float64)
    for m in range(8):
        for k in range(8):
            g[2 * m + k] += float(f1[m]) * float(s0[k])
    return g.astype(np.float32)


def _make_fused_parity_stationaries(s0, s1):
    """Levels 0+1 fused (d1, a1 direct from x): 6 mats
    [MA0, MB0, C0, MA1, MB1, C1], 64-output parity blocks, stride 4."""
    u = _composite(s0, s1)
    v = _composite(s0, _wavelet(s1))
    mats = np.zeros((6, 128, 128), dtype=np.float32)
    for pi in (0, 1):
        MA, MB, C = mats[3 * pi], mats[3 * pi + 1], mats[3 * pi + 2]
        for m in range(128):
            a_out = (m < 64) == (pi == 0)
            q = m % 64
            g = u if a_out else v
            for k in range(22):
                i = 4 * q - k
                if 0 <= i < 128:
                    MA[i, m] = g[k]
                elif i >= 128:
                    MB[i - 128, m] = g[k]
                else:
                    C[i + 128, m] = g[k]
    return mats


def _make_wmat(scaling):
    """[9 fused mats (d0 W1,W2,Wc + d1a1 MA0,MB0,C0,MA1,MB1,C1)]
    + [4 parity mats per level for levels 2..LEVELS-1]."""
    s0 = np.asarray(scaling[0], dtype=np.float32)
    s1 = np.asarray(scaling[1], dtype=np.float32)
    mats = [_make_stationaries(_wavelet(s0)),
            _make_fused_parity_stationaries(s0, s1)]
    for lvl in range(2, LEVELS):
        mats.append(_make_parity_stationaries(
            np.asarray(scaling[lvl], dtype=np.float32)))
    allw = np.concatenate(mats, axis=0)
    return np.ascontiguousarray(allw.transpose(1, 0, 2).reshape(128, -1))


def _round_f32r(arr):
    """Round fp32 to the nearest FP32R value (1s/8e/11m; low 12 bits zero)."""
    u = np.ascontiguousarray(arr, dtype=np.float32).view(np.uint32)
    r = (u + 0x7FF + ((u >> 12) & 1)) & np.uint32(0xFFFFF000)
    return r.view(np.float32)


def _pack_x_shard(x_rows):
    rows, n = x_rows.shape
    nb = n // 128
    blocks = x_rows.astype(np.float16).reshape(rows, nb, 128).transpose(2, 0, 1)
    xt = np.empty((128, rows, nb + 1), dtype=np.float16)
    xt[:, :, 1:] = blocks
    xt[:, :, 0] = blocks[:, :, nb - 1]           # circular halo column
    return np.ascontiguousarray(xt.reshape(128, rows * (nb + 1)))


def _unpack_blocks(arr, rows):
    """[128, rows*nob] natural block layout -> [rows, nob*128]."""
    nob = arr.shape[1] // rows
    return arr.reshape(128, rows, nob).transpose(1, 2, 0).reshape(rows, nob * 128)


def _unpack_d_parity(arr, rows):
    """Parity-packed detail layout -> [rows, n/2].

    arr [128, rows*nbh]: partition 64+q col (r, cb) = d[r, 128cb + q]
    (even output column), partition q = d[r, 128cb + 64 + q] (odd column).
    """
    nbh = arr.shape[1] // rows
    a3 = arr.reshape(128, rows, nbh)
    out = np.empty((rows, nbh, 2, 64), dtype=arr.dtype)
    out[:, :, 0, :] = a3[64:128].transpose(1, 2, 0)
    out[:, :, 1, :] = a3[0:64].transpose(1, 2, 0)
    return out.reshape(rows, nbh * 128)


def _is_orthonormal_qmf(scaling):
    s = np.asarray(scaling, dtype=np.float64)
    if s.shape != (LEVELS, 8):
        return False
    for lvl in range(LEVELS):
        f = s[lvl]
        for m in range(4):
            v = np.dot(f[: 8 - 2 * m], f[2 * m:])
            if abs(v - (1.0 if m == 0 else 0.0)) > 1e-4:
                return False
    return True


def _dwt_backward_numpy(ds, a, scaling):
    """Fallback inverse transform (float64 FFT) for non-orthonormal filters."""
    a = np.asarray(a, dtype=np.float64)
    for lvl in reversed(range(LEVELS)):
        s = np.asarray(scaling[lvl], dtype=np.float64)
        w = _wavelet(s)
        d = np.asarray(ds[lvl], dtype=np.float64)
        n = d.shape[-1] * 2
        fd = np.zeros((d.shape[0], n))
        fd[:, ::2] = d
        fa = np.zeros((a.shape[0], n))
        fa[:, ::2] = a
        a = (np.fft.irfft(np.fft.rfft(fd, axis=-1)
                          * np.conj(np.fft.rfft(w, n=n)), n=n, axis=-1)
             + np.fft.irfft(np.fft.rfft(fa, axis=-1)
                            * np.conj(np.fft.rfft(s, n=n)), n=n, axis=-1))
    return a


# ----------------------------- device kernel ------------------------------

def _build_dwt(tc, xt, wmat, d_outs, a_out, n0=N0, rows=ROWS, levels=LEVELS,
               rg_rows=RG_ROWS):
    """Forward DWT: levels 0+1 fused (d0 directly; d1/a1 via 22-tap stride-4
    composite filters), level 2 row-grouped, levels 3+ merged.  Row-groups
    are wavefront-interleaved so input streaming spreads across the run.
    """
    nc = tc.nc
    nb0 = n0 // 128
    n_rg = rows // rg_rows
    NWF = 9  # fused-section stationary count
    with ExitStack() as ctx:
        wpool = ctx.enter_context(tc.tile_pool(name="wpool", bufs=1))
        x0pool = ctx.enter_context(tc.tile_pool(name="x0pool", bufs=4))
        xpool = ctx.enter_context(tc.tile_pool(name="xpool", bufs=2))
        x1pool = ctx.enter_context(tc.tile_pool(name="x1pool", bufs=1))
        stpool = ctx.enter_context(tc.tile_pool(name="stpool", bufs=2))
        p0pool = ctx.enter_context(tc.tile_pool(name="p0pool", bufs=3, space="PSUM"))
        p1pool = ctx.enter_context(tc.tile_pool(name="p1pool", bufs=3, space="PSUM"))
        pdpool = ctx.enter_context(tc.tile_pool(name="pdpool", bufs=2, space="PSUM"))

        W = wpool.tile([128, (NWF + (levels - 2) * 4) * 128], F16, name="Wsb")
        w_loaded = set()

        def load_w(sec):
            if sec in w_loaded:
                return
            w_loaded.add(sec)
            if sec == "f":
                nc.sync.dma_start(W[:, 0:NWF * 128], wmat[:, 0:NWF * 128])
            else:
                k0 = (NWF + (sec - 2) * 4) * 128
                nc.sync.dma_start(W[:, k0:k0 + 512], wmat[:, k0:k0 + 512])

        def woff(lvl):
            return (NWF + (lvl - 2) * 4) * 128

        xt3 = xt.rearrange("p (r b) -> p r b", b=nb0 + 1)
        Xs = {}
        halo_done = set()

        def do_fused(rg):
            """Levels 0 and 1 for rows [rg*rg_rows, (rg+1)*rg_rows)."""
            load_w("f")
            nb = nb0
            nob0 = nb // 2        # 128-wide d0 blocks per row
            nbh1 = nb // 4        # 64-wide d1/a1 parity columns per row
            CH = 4                # rows per chunk
            dh0 = d_outs[0].rearrange("p (r c) -> p r c", c=nob0)
            dh1 = d_outs[1].rearrange("p (r c) -> p r c", c=nbh1)
            X2 = xpool.tile([128, rg_rows, nbh1 + 1], F16, name=f"X2_{rg}",
                            tag="X2")
            Xs[(rg, 2)] = X2
            Wd = [W[:, i * 128:(i + 1) * 128] for i in range(3)]
            Fm = [W[:, (3 + i) * 128:(4 + i) * 128] for i in range(6)]
            stf = stpool.tile([128, rg_rows, nbh1], F16, tag="stf", name="stf")
            for ch in range(rg_rows // CH):
                r0 = ch * CH
                g0 = rg * rg_rows + r0
                x0t = x0pool.tile([128, CH, nb + 1], F16, tag="x0t", name="x0t")
                nc.sync.dma_start(x0t[:], xt3[:, g0:g0 + CH, :])
                # ---- d0: 128-wide blocks, two row-pairs ----
                std0 = stpool.tile([128, CH, nob0], F16, tag="std0", name="std0")
                for h in (0, 1):
                    rs2 = slice(2 * h, 2 * h + 2)
                    pd0 = pdpool.tile([128, 2, nob0], F32, tag="pd0", name="pd0")
                    nc.tensor.matmul(pd0[:], Wd[0], x0t[:, rs2, 1:nb:2],
                                     start=True, stop=False)
                    nc.tensor.matmul(pd0[:], Wd[1], x0t[:, rs2, 2:nb + 1:2],
                                     start=False, stop=False)
                    nc.tensor.matmul(pd0[:], Wd[2], x0t[:, rs2, 0:nb - 1:2],
                                     start=False, stop=True)
                    if h == 0:
                        nc.vector.tensor_copy(std0[:, rs2, :], pd0[:])
                    else:
                        nc.scalar.copy(std0[:, rs2, :], pd0[:])
                # ---- d1/a1 fused: 64-wide parity blocks, stride 4 ----
                pf0 = p0pool.tile([128, CH, nbh1], F32, tag="ps0", name="pf0")
                pf1 = p1pool.tile([128, CH, nbh1], F32, tag="ps1", name="pf1")
                nc.tensor.matmul(pf0[:], Fm[0], x0t[:, :, 1:nb:4],
                                 start=True, stop=False)
                nc.tensor.matmul(pf1[:], Fm[3], x0t[:, :, 3:nb:4],
                                 start=True, stop=False)
                nc.tensor.matmul(pf0[:], Fm[1], x0t[:, :, 2:nb:4],
                                 start=False, stop=False)
                nc.tensor.matmul(pf1[:], Fm[4], x0t[:, :, 4:nb + 1:4],
                                 start=False, stop=False)
                nc.tensor.matmul(pf1[:], Fm[5], x0t[:, :, 2:nb:4],
                                 start=False, stop=True)
                nc.tensor.matmul(pf0[:], Fm[2], x0t[:, :, 0:nb - 1:4],
                                 start=False, stop=True)
                wr = slice(r0, r0 + CH)
                nc.vector.tensor_copy(X2[0:64, wr, 1:1 + nbh1], pf0[0:64, :, :])
                nc.scalar.copy(X2[64:128, wr, 1:1 + nbh1], pf1[64:128, :, :])
                nc.vector.tensor_copy(stf[0:64, wr, :], pf1[0:64, :, :])
                nc.scalar.copy(stf[64:128, wr, :], pf0[64:128, :, :])
                nc.sync.dma_start(dh0[:, g0:g0 + CH, :], std0[:])
            r0g = rg * rg_rows
            nc.sync.dma_start(dh1[:, r0g:r0g + rg_rows, :], stf[:])

        def do_unit(rg, lvl):
            """Levels >= 2; lvl 2 per-rowgroup, lvl >= 3 all rows."""
            fine = lvl == 2
            row0 = rg * rg_rows if fine else 0
            nrows = rg_rows if fine else rows
            nb = (n0 >> lvl) // 128
            nbh = nb // 2
            nr = min(nrows, max(1, 512 // nbh))
            nchunks = nrows // nr
            sc = min(SC_MAX, nchunks)
            last = lvl + 1 == levels
            load_w(lvl)
            dh = d_outs[lvl].rearrange("p (r c) -> p r c", c=nbh)
            if last:
                ah = a_out.rearrange("p (r c) -> p r c", c=nbh)
            else:
                nkey = ("all", lvl + 1)
                if nkey not in Xs:
                    Xs[nkey] = x1pool.tile([128, rows, nbh + 1], F16,
                                           name=f"X{lvl + 1}_all",
                                           tag=f"X{lvl + 1}")
                Xn = Xs[nkey]

            key = (rg, 2) if fine else ("all", lvl)
            Xl = Xs[key]
            if key not in halo_done:
                halo_done.add(key)
                nc.vector.tensor_copy(Xl[:, :, 0:1], Xl[:, :, nb:nb + 1])

            k0 = woff(lvl)
            M0, C0 = W[:, k0:k0 + 128], W[:, k0 + 128:k0 + 256]
            M1, C1 = W[:, k0 + 256:k0 + 384], W[:, k0 + 384:k0 + 512]

            st = sta = None
            for ch in range(nchunks):
                r0 = ch * nr
                g0 = row0 + r0
                rs = slice(r0, r0 + nr)
                Xc = Xl
                ps0 = p0pool.tile([128, nr, nbh], F32, tag="ps0", name="ps0")
                ps1 = p1pool.tile([128, nr, nbh], F32, tag="ps1", name="ps1")
                nc.tensor.matmul(ps0[:], M0, Xc[:, rs, 1:nb:2],
                                 start=True, stop=False)
                nc.tensor.matmul(ps1[:], M1, Xc[:, rs, 2:nb + 1:2],
                                 start=True, stop=False)
                nc.tensor.matmul(ps1[:], C1, Xc[:, rs, 1:nb:2],
                                 start=False, stop=True)
                nc.tensor.matmul(ps0[:], C0, Xc[:, rs, 0:nb - 1:2],
                                 start=False, stop=True)

                sci = ch % sc
                ss = slice(sci * nr, (sci + 1) * nr)
                if sci == 0:
                    st = stpool.tile([128, sc * nr, nbh], F16, tag="st",
                                     name="st")
                    if last:
                        sta = stpool.tile([128, sc * nr, nbh], F16, tag="sta",
                                          name="sta")
                if not last:
                    wr = slice(g0, g0 + nr)
                    nc.vector.tensor_copy(Xn[0:64, wr, 1:1 + nbh],
                                          ps0[0:64, :, :])
                    nc.scalar.copy(Xn[64:128, wr, 1:1 + nbh],
                                   ps1[64:128, :, :])
                else:
                    nc.vector.tensor_copy(sta[0:64, ss, :], ps0[0:64, :, :])
                    nc.scalar.copy(sta[64:128, ss, :], ps1[64:128, :, :])
                nc.vector.tensor_copy(st[0:64, ss, :], ps1[0:64, :, :])
                nc.scalar.copy(st[64:128, ss, :], ps0[64:128, :, :])

                if sci == sc - 1:
                    d0 = row0 + (ch - sci) * nr
                    nc.sync.dma_start(dh[:, d0:d0 + sc * nr, :], st[:])
                    if last:
                        nc.sync.dma_start(ah[:, d0:d0 + sc * nr, :], sta[:])

        # wavefront: fused rowgroups interleaved with level-2 units
        order = []
        for step in range(n_rg + 1):
            if step < n_rg:
                order.append(("f", step))
            if step >= 1:
                order.append((2, step - 1))
        for lvl in range(3, levels):
            order.append((lvl, 0))
        for kind, rg in order:
            if kind == "f":
                do_fused(rg)
            else:
                do_unit(rg, kind)


_MODULE_CACHE = {}


def _get_module():
    if "nc" in _MODULE_CACHE:
        return _MODULE_CACHE["nc"]
    nc = bacc.Bacc("TRN2", target_bir_lowering=False, debug=False,
                   num_devices=N_CORES)
    xt = nc.dram_tensor("xt", [128, ROWS * (N0 // 128 + 1)], F16,
                        kind="ExternalInput").ap()
    wmat = nc.dram_tensor("wmat", [128, (9 + (LEVELS - 2) * 4) * 128], F16,
                          kind="ExternalInput").ap()
    d_outs = []
    for lvl in range(LEVELS):
        nbh = (N0 >> lvl) // 256
        d_outs.append(nc.dram_tensor(f"d{lvl}", [128, ROWS * nbh], F16,
                                     kind="ExternalOutput").ap())
    a_out = nc.dram_tensor("aF", [128, ROWS * ((N0 >> (LEVELS - 1)) // 256)],
                           F16, kind="ExternalOutput").ap()
    with tile.TileContext(nc) as tc:
        _build_dwt(tc, xt, wmat, d_outs, a_out)
    nc.compile()
    _MODULE_CACHE["nc"] = nc
    return nc


def run(x, scaling, **spmd_kwargs):
    """Full pipeline.  Returns (denoised, coeffs, BassKernelResults)."""
    x = np.ascontiguousarray(np.asarray(x, dtype=np.float32))
    scaling = np.asarray(scaling, dtype=np.float32)
    assert x.shape == (N_ROWS, N0), x.shape
    assert scaling.shape == (LEVELS, 8), scaling.shape

    nc = _get_module()
    wmat = _make_wmat(scaling).astype(np.float16)
    in_maps = []
    for c in range(N_CORES):
        in_maps.append({
            "xt": _pack_x_shard(x[c * ROWS:(c + 1) * ROWS]),
            "wmat": wmat,
        })

    res = run_bass_kernel_spmd(nc, in_maps, core_ids=list(range(N_CORES)),
                               **spmd_kwargs)

    coeffs = np.empty((N_ROWS, N0), dtype=np.float32)
    off = 0
    ds_full = []
    for lvl in range(LEVELS):
        half = (N0 >> lvl) // 2
        dcols = coeffs[:, off:off + half]
        unpack = _unpack_blocks if lvl == 0 else _unpack_d_parity
        for c in range(N_CORES):
            dcols[c * ROWS:(c + 1) * ROWS] = unpack(
                res.results[c][f"d{lvl}"], ROWS).astype(np.float32)
        ds_full.append(dcols)
        off += half
    a_full = np.empty((N_ROWS, N0 - off), dtype=np.float32)
    for c in range(N_CORES):
        a_full[c * ROWS:(c + 1) * ROWS] = _unpack_blocks(
            res.results[c]["aF"], ROWS).astype(np.float32)
    coeffs[:, off:] = a_full

    if _is_orthonormal_qmf(scaling):
        # Orthonormal QMF bank + untouched coefficients => the inverse
        # transform is exactly the identity (reference pad is a no-op).
        denoised = x.copy()
    else:
        denoised = _dwt_backward_numpy(ds_full, a_full, scaling).astype(np.float32)

    return denoised, coeffs, res


def kernel(x, scaling):
    denoised, coeffs, _ = run(x, scaling)
    return denoised, coeffs
